# revision 1
# baseline (speedup 1.0000x reference)
"""Trainium2 Bass kernel v3 for the decoder LSTM (B=256, T=2048, HID=128, OUT=6).

Data-parallel over batch (8 cores x 32 lanes). Single recurrence chain per
core, latency-optimized (the workload is serial-latency-bound: 2048 LSTM
steps at ~2.06us each, no engine above ~42% busy). Per-step chain:

  PE 4 gate matmuls (order g,i,f,o; g-gate table/W rows pre-scaled 2x on the
  host so tanh(g) = 2*sigmoid(2g)-1 comes out of sigmoid algebra) -> ACT
  sigmoid over [2g|i] as soon as matmul i lands, then over [f|o] (two
  2-region instructions instead of four) -> DVE cell update with bf16
  intermediates -> ACT tanh(c) -> DVE h -> PE.  Measured 4.213 ms on trn2.

Off the critical path: the one-hot input-projection prefill for the NEXT
PSUM group is split into 8 half-size matmuls, one per step, so no prefill
burst ever sits ahead of a critical gate matmul in the PE queue; the fc
logits accumulate in a PSUM window bank for 64 steps (bias added by a K=1
ones-matmul once per window) and are evacuated by a single ACT copy.
"""

import os
import sys

for _p in ("/opt/trn_rl_repo", "/root/.axon_site/_ro/trn_rl_repo"):
    if os.path.isdir(_p) and _p not in sys.path:
        sys.path.insert(0, _p)

import numpy as np

B, T, VOCAB, EMB, HID, OUT = 256, 2048, 7, 20, 128, 6
NCORES = 8
BL = B // NCORES  # 32 lanes per core
G = 8  # steps per gate-PSUM group (psA=[g|i], psB=[f|o], 512 f32 each)
FCW = 64  # steps per fc PSUM window ([32, 384] f32)
GI, GF, GG, GO = 0, 1, 2, 3  # PyTorch gate order in W_hh rows / table cols


def _split_overloaded_waits(nc, mybir, max_other=1):
    """walrus in this env rejects instructions with more than a couple of sem
    waits (and InstDrain with any). Move excess waits onto same-engine NoOps
    emitted just before; same-engine program order preserves semantics."""
    n_split = 0
    for f in nc.m.functions:
        for blk in f.blocks:
            out = []
            changed = False
            for inst in blk.instructions:
                si = inst.sync_info
                waits = list(si.on_wait) if si is not None and si.on_wait else []
                limit = 0 if isinstance(inst, mybir.InstDrain) else max_other
                if len(waits) > limit:
                    moved = waits if limit == 0 else waits[limit:]
                    keep = [] if limit == 0 else waits[:limit]
                    for i0, w in enumerate(moved):
                        nop = mybir.InstNoOp(
                            name=f"{inst.name}-wsplit{i0}", ins=[], outs=[]
                        )
                        nop.engine = inst.engine
                        nop.sync_info = mybir.SyncInfo(on_wait=[w], on_update=[])
                        out.append(nop)
                        n_split += 1
                    inst.sync_info = mybir.SyncInfo(
                        on_wait=keep,
                        on_update=list(si.on_update) if si.on_update else [],
                    )
                    changed = True
                out.append(inst)
            if changed:
                blk.instructions = out
    return n_split


def _patch_tile_drain():
    import concourse.tile as tile
    from concourse.vector_clock import ScopedClock, VectorClock

    def _drain_and_barrier_split(self, tick_clock, wait_clock):
        gc = tick_clock.global_clock
        n = len(gc)
        for j in range(n):
            if gc[j] <= 0:
                continue
            vec = [0] * n
            vec[j] = gc[j]
            nop = self.nc.sync.nop(nofuse=True, hint=f"drain_split_{j}")
            wait_clock.add_sem_waits(nop.ins, ScopedClock({None: VectorClock(vec)}))
        self.nc.sync.drain()
        self.nc.all_engine_barrier()
        assert self.sems is not None
        popped = self.nc._tile_sem_poison_stack.pop()
        assert popped is self._sem_poison
        self.nc.clear_and_free_semaphores(list(self.sems.allocated().values()))
        self.nc.all_engine_barrier()

    tile.TileContext._drain_and_barrier = _drain_and_barrier_split


_BUILD_CACHE = {}


def _build_nc(t_steps, for_sim=False):
    tail_mode = os.environ.get("V3_TAIL", "act")  # pade | act | skip
    fc_mode = os.environ.get("V3_FC", "win")       # win | off
    pre_mode = os.environ.get("V3_PRE", "slice")   # slice | group
    act_n = int(os.environ.get("V3_ACT", "2"))     # 4 | 3 | 2 fused (host gscale)
    bf_mid = os.environ.get("V3_BF", "1") == "1"
    bf2 = os.environ.get("V3_BF", "0") == "2"
    hoist = os.environ.get("V3_HOIST", "0") == "1"
    split_h = os.environ.get("V3_SPLITH", "0") == "1"
    pool_m = os.environ.get("V3_POOLM", "0") == "1"
    key = (t_steps, for_sim, tail_mode, fc_mode, pre_mode, act_n, pool_m, split_h, bf_mid, hoist, bf2)
    if key in _BUILD_CACHE:
        return _BUILD_CACHE[key]
    import concourse.bass as bass
    import concourse.mybir as mybir
    import concourse.tile as tile

    _patch_tile_drain()

    assert t_steps % G == 0 and t_steps % FCW == 0
    f32 = mybir.dt.float32
    bf16 = mybir.dt.bfloat16
    AF = mybir.ActivationFunctionType
    ALU = mybir.AluOpType
    n_groups = t_steps // G

    nc = bass.Bass("TRN2", target_bir_lowering=False, debug=False)
    d_oh = nc.dram_tensor("onehot", [VOCAB, t_steps * BL], bf16, kind="ExternalInput")
    d_c0 = nc.dram_tensor("c0T", [HID, BL], f32, kind="ExternalInput")
    d_w = nc.dram_tensor("w", [HID, 4 * HID], bf16, kind="ExternalInput")
    d_tbl = nc.dram_tensor("tbl", [VOCAB, 4 * HID], bf16, kind="ExternalInput")
    d_wfc = nc.dram_tensor("wfc", [HID, OUT], bf16, kind="ExternalInput")
    d_bfcw = nc.dram_tensor("bfcw", [1, FCW * OUT], bf16, kind="ExternalInput")
    d_out = nc.dram_tensor("out", [BL, t_steps, OUT], f32, kind="ExternalOutput")

    with tile.TileContext(nc) as tc, tc.tile_pool(name="const", bufs=1) as constp:
        w_sb = constp.tile([HID, 4 * HID], bf16, name="w_sb")
        tbl_sb = constp.tile([VOCAB, 4 * HID], bf16, name="tbl_sb")
        wfc_sb = constp.tile([HID, OUT], bf16, name="wfc_sb")
        bfcw_sb = constp.tile([1, FCW * OUT], bf16, name="bfcw_sb")
        ones_sb = constp.tile([1, BL], bf16, name="ones_sb")
        cst = constp.tile([HID, BL], f32, name="cst")
        h0_sb = constp.tile([HID, BL], bf16, name="h0_sb")
        scr = constp.tile([HID, BL], bf16, name="scr")
        logit_sb = constp.tile([BL, t_steps * OUT], f32, name="logit_sb")
        den_sb = constp.tile([BL, t_steps], f32, name="den_sb")

        nc.sync.dma_start(w_sb[:], d_w.ap())
        nc.sync.dma_start(tbl_sb[:], d_tbl.ap())
        nc.sync.dma_start(wfc_sb[:], d_wfc.ap())
        nc.sync.dma_start(bfcw_sb[:], d_bfcw.ap())
        nc.sync.dma_start(cst[:], d_c0.ap())
        nc.vector.memset(h0_sb[:], 0.0)
        nc.vector.memset(ones_sb[:], 1.0)
        # Pin the sigmoid_and_others table (contains tanh too) before the loop.
        nc.scalar.activation(scr[:], h0_sb[:], AF.Sigmoid)

        with (
            tc.tile_pool(name="ohp", bufs=4) as ohp,
            tc.tile_pool(name="ringp", bufs=4) as ringp,
            tc.tile_pool(name="gatep", bufs=3, space="PSUM") as gatep,
            tc.tile_pool(name="fcp", bufs=2, space="PSUM") as fcp,
            tc.tile_pool(name="workp", bufs=4) as workp,
        ):
            half = G * BL  # 256 columns per gate block

            def alloc_group(g):
                """DMA the group's one-hot block and allocate its PSUM tiles."""
                oh = ohp.tile([VOCAB, G * BL], bf16, tag="oh")
                nc.sync.dma_start(
                    oh[:], d_oh.ap()[:, g * G * BL : (g + 1) * G * BL]
                )
                psA = gatep.tile([128, 2 * half], f32, tag="psA")
                psB = gatep.tile([128, 2 * half], f32, tag="psB")
                return (oh, psA, psB)

            def prefill_slice(tiles, j):
                """One half-gate prefill matmul (128 columns = 4 steps)."""
                oh, psA, psB = tiles
                ps = psA if j < 4 else psB
                q = (GG, GG, GI, GI, GF, GF, GO, GO)[j]
                h0c = (j % 2) * 128  # first/second half of the gate block
                b0 = (j // 2 % 2) * half  # gate block base within the tile
                nc.tensor.matmul(
                    ps[:, b0 + h0c : b0 + h0c + 128],
                    tbl_sb[:, q * HID : (q + 1) * HID],
                    oh[:, h0c : h0c + 128],
                    start=(j % 4 == 0),
                    stop=False,
                    skip_group_check=True,
                )

            cur = alloc_group(0)
            for j in range(8):
                prefill_slice(cur, j)
            nxt = None
            if pre_mode == "group":
                if n_groups > 1:
                    nxt = alloc_group(1)
                    for j in range(8):
                        prefill_slice(nxt, j)

            fcw_tile = None
            pending_fc = None  # h slot of the previous step

            def flush_fc(t):
                nonlocal fcw_tile
                w0 = t % FCW
                if w0 == 0:
                    fcw_tile = fcp.tile([BL, FCW * OUT], f32, tag="fcw")
                    nc.tensor.matmul(
                        fcw_tile[:], ones_sb[:], bfcw_sb[:],
                        start=True, stop=False,
                    )
                nc.tensor.matmul(
                    fcw_tile[:, w0 * OUT : (w0 + 1) * OUT],
                    pending_fc, wfc_sb[:],
                    start=False, stop=(w0 == FCW - 1),
                )
                if w0 == FCW - 1:
                    wb = t - (FCW - 1)
                    nc.scalar.copy(
                        logit_sb[:, wb * OUT : (t + 1) * OUT], fcw_tile[:]
                    )

            for g in range(n_groups):
                oh, psA, psB = cur
                for s in range(G):
                    t = g * G + s
                    h_prev = h0_sb[:] if t == 0 else h_prev_slot
                    if hoist and t > 0:
                        # tiny matmul reading h first: carries the h sem-wait
                        # so the gate matmuls' LDWEIGHTS dispatch waitless
                        dmy = dumpp.tile([1, 1], f32, tag="dmy")
                        nc.tensor.matmul(
                            dmy[:], h_prev[:, 0:1], h_prev[:, 0:1],
                            start=True, stop=True,
                        )
                    last = s == G - 1
                    cA = s * BL
                    cB = half + s * BL
                    # gate pre-activations, order g, i, f, o
                    nc.tensor.matmul(
                        psA[:, cA : cA + BL], w_sb[:, GG * HID : (GG + 1) * HID],
                        h_prev, start=False, stop=False,
                        skip_group_check=True,
                    )
                    nc.tensor.matmul(
                        psA[:, cB : cB + BL], w_sb[:, GI * HID : (GI + 1) * HID],
                        h_prev, start=False, stop=last,
                        skip_group_check=True,
                    )
                    nc.tensor.matmul(
                        psB[:, cA : cA + BL], w_sb[:, GF * HID : (GF + 1) * HID],
                        h_prev, start=False, stop=False,
                        skip_group_check=True,
                    )
                    nc.tensor.matmul(
                        psB[:, cB : cB + BL], w_sb[:, GO * HID : (GO + 1) * HID],
                        h_prev, start=False, stop=last,
                        skip_group_check=True,
                    )
                    # non-critical PE work rides behind the gate matmuls:
                    # the previous step's fc, and one prefill slice for the
                    # next group.
                    if pending_fc is not None:
                        if fc_mode != "off":
                            flush_fc(t - 1)
                        pending_fc = None
                    if pre_mode == "slice" and g + 1 < n_groups:
                        if s == 0:
                            nxt = alloc_group(g + 1)
                        prefill_slice(nxt, s)
                    # per-gate activations fire as each matmul lands
                    tg = workp.tile([HID, BL], bf16, tag="tg")
                    si = workp.tile([HID, BL], bf16, tag="si")
                    sf = workp.tile([HID, BL], bf16, tag="sf")
                    so = workp.tile([HID, BL], bf16, tag="so")
                    if act_n == 4:
                        nc.scalar.activation(tg[:], psA[:, cA : cA + BL], AF.Tanh)
                        nc.scalar.activation(si[:], psA[:, cB : cB + BL], AF.Sigmoid)
                        nc.scalar.activation(sf[:], psB[:, cA : cA + BL], AF.Sigmoid)
                        nc.scalar.activation(so[:], psB[:, cB : cB + BL], AF.Sigmoid)
                    elif act_n == 5:
                        sfo = workp.tile([HID, 2 * BL], bf16, tag="sfo")
                        pb2 = psB[:].rearrange("p (two g b) -> p two g b", two=2, g=G)
                        sfo3 = sfo[:].rearrange("p (two b) -> p two b", two=2)
                        nc.scalar.activation(tg[:], psA[:, cA : cA + BL], AF.Tanh)
                        nc.scalar.activation(si[:], psA[:, cB : cB + BL], AF.Sigmoid)
                        nc.scalar.activation(sfo3, pb2[:, :, s, :], AF.Sigmoid)
                        sf = sfo[:, 0:BL]
                        so = sfo[:, BL : 2 * BL]
                    elif act_n == 3:
                        sfo = workp.tile([HID, 2 * BL], bf16, tag="sfo")
                        pb2 = psB[:].rearrange("p (two g b) -> p two g b", two=2, g=G)
                        sfo3 = sfo[:].rearrange("p (two b) -> p two b", two=2)
                        nc.scalar.activation(tg[:], psA[:, cA : cA + BL], AF.Tanh)
                        nc.scalar.activation(si[:], psA[:, cB : cB + BL], AF.Sigmoid)
                        nc.scalar.activation(sfo3, pb2[:, :, s, :], AF.Sigmoid)
                        sf = sfo[:, 0:BL]
                        so = sfo[:, BL : 2 * BL]
                    else:
                        # psA g-half holds 2*g (host pre-scales table+W rows);
                        # sgi = [sigmoid(2g), sigmoid(i)] in one 2-region instr
                        sgi = workp.tile([HID, 2 * BL], bf16, tag="sgi")
                        sfo = workp.tile([HID, 2 * BL], bf16, tag="sfo")
                        pa2 = psA[:].rearrange("p (two g b) -> p two g b", two=2, g=G)
                        pb2 = psB[:].rearrange("p (two g b) -> p two g b", two=2, g=G)
                        sgi3 = sgi[:].rearrange("p (two b) -> p two b", two=2)
                        sfo3 = sfo[:].rearrange("p (two b) -> p two b", two=2)
                        nc.scalar.activation(sgi3, pa2[:, :, s, :], AF.Sigmoid)
                        nc.scalar.activation(sfo3, pb2[:, :, s, :], AF.Sigmoid)
                        si = sgi[:, BL : 2 * BL]
                        sf = sfo[:, 0:BL]
                        so = sfo[:, BL : 2 * BL]
                    # cell update + inline Pade(5,4) tanh(c), all on DVE
                    ig = workp.tile([HID, BL], f32, tag="ig")
                    mm = workp.tile([HID, BL], f32, tag="mm")
                    y2 = workp.tile([HID, BL], f32, tag="y2")
                    n1 = workp.tile([HID, BL], f32, tag="n1")
                    t2 = workp.tile([HID, BL], f32, tag="t2")
                    num = workp.tile([HID, BL], f32, tag="num")
                    dn = workp.tile([HID, BL], f32, tag="dn")
                    rc = workp.tile([HID, BL], f32, tag="rc")
                    t0 = workp.tile([HID, BL], f32, tag="t0")
                    igb = workp.tile([HID, BL], bf16, tag="igb")
                    mmb = workp.tile([HID, BL], bf16, tag="mmb")
                    if act_n == 5:
                        # ig(bf16) = si * tg ; m(bf16) = sf * c ; c' = m + ig
                        nc.vector.tensor_mul(igb[:], si[:], tg[:])
                        nc.vector.tensor_mul(mmb[:], sf, cst[:])
                        nc.vector.tensor_add(cst[:], mmb[:], igb[:])
                    elif act_n == 4 or act_n == 3:
                        si_ap = si[:] if act_n == 4 else si[:]
                        nc.vector.tensor_mul(ig[:], si_ap, tg[:])
                    elif bf2:
                        # every tensor_tensor arranged as (in0 2-byte, in1
                        # 4-byte) -> the fast DVE read mode seen on the m op
                        u2f = workp.tile([HID, BL], f32, tag="u2f")
                        ignf = workp.tile([HID, BL], f32, tag="ignf")
                        nc.vector.scalar_tensor_tensor(
                            u2f[:], sgi[:, 0:BL], 2.0, si,
                            op0=ALU.mult, op1=ALU.mult,
                        )
                        nc.vector.tensor_mul(mmb[:], sf, cst[:])
                        nc.vector.tensor_sub(ignf[:], si, u2f[:])
                        nc.vector.tensor_sub(cst[:], mmb[:], ignf[:])
                    elif not bf_mid:
                        # ig = si*(2*sigmoid(2g)-1) = (2*sghat)*si - si
                        u2 = workp.tile([HID, BL], f32, tag="u2")
                        nc.vector.scalar_tensor_tensor(
                            u2[:], sgi[:, 0:BL], 2.0, si,
                            op0=ALU.mult, op1=ALU.mult,
                        )
                        nc.vector.tensor_sub(ig[:], u2[:], si)
                    else:
                        # bf16 intermediates, src0 kept 2-byte for DVE 2x:
                        # ign = si - 2*sghat*si = -ig ; m = sf*c (bf16)
                        # c' = m - ign
                        u2b = workp.tile([HID, BL], bf16, tag="u2b")
                        ignb = workp.tile([HID, BL], bf16, tag="ignb")
                        nc.vector.scalar_tensor_tensor(
                            u2b[:], sgi[:, 0:BL], 2.0, si,
                            op0=ALU.mult, op1=ALU.mult,
                        )
                        # m between u2 and ign: hides u2's write-pipeline
                        # latency from ign (and ign's from c').
                        nc.vector.tensor_mul(mmb[:], sf, cst[:])
                        nc.vector.tensor_sub(ignb[:], si, u2b[:])
                        nc.vector.tensor_sub(cst[:], mmb[:], ignb[:])
                    if act_n != 5 and not (act_n == 2 and (bf_mid or bf2)):
                        sf_ap = sf[:] if act_n == 4 else sf
                        if pool_m:
                            nc.gpsimd.tensor_mul(mm[:], sf_ap, cst[:])
                        else:
                            nc.vector.tensor_mul(mm[:], sf_ap, cst[:])
                        nc.vector.tensor_add(cst[:], mm[:], ig[:])
                    hslot = ringp.tile([HID, BL], bf16, tag="h")
                    if tail_mode == "pade":
                        nc.vector.tensor_mul(y2[:], cst[:], cst[:])
                        nc.vector.scalar_tensor_tensor(
                            n1[:], y2[:], 105.0, y2[:], op0=ALU.add, op1=ALU.mult
                        )
                        nc.vector.tensor_scalar(
                            t2[:], y2[:], 1155.0, 945.0, op0=ALU.mult, op1=ALU.subtract
                        )
                        nc.vector.scalar_tensor_tensor(
                            num[:], n1[:], 945.0, cst[:], op0=ALU.add, op1=ALU.mult
                        )
                        nc.vector.scalar_tensor_tensor(
                            dn[:], n1[:], 15.0, t2[:], op0=ALU.mult, op1=ALU.subtract
                        )
                        nc.vector.reciprocal_approx_fast(rc[:], dn[:])
                        nc.vector.tensor_mul(t0[:], num[:], rc[:])
                        nc.vector.tensor_mul(hslot[:], t0[:], so if act_n != 4 else so[:])
                    elif tail_mode == "act":
                        tcl = workp.tile([HID, BL], f32 if bf2 else bf16, tag="tcl")
                        nc.scalar.activation(tcl[:], cst[:], AF.Tanh)
                        nc.vector.tensor_mul(hslot[:], so if act_n != 4 else so[:], tcl[:])
                    else:  # skip (timing-ablation only; wrong values)
                        nc.vector.tensor_mul(hslot[:], so if act_n != 4 else so[:], cst[:])
                    h_prev_slot = hslot[:]
                    pending_fc = hslot[:]
                if g + 1 < n_groups:
                    cur = nxt
                    if pre_mode == "group" and g + 2 < n_groups:
                        nxt = alloc_group(g + 2)
                        for j in range(8):
                            prefill_slice(nxt, j)
            if fc_mode != "off":
                flush_fc(t_steps - 1)
            pending_fc = None

        # ---- phase 2: softmax over OUT, windowed so ACT (exp), DVE
        # (reduce/recip/mul) and the output DMA pipeline against each other;
        # one exp-table load at the first window. ----
        p3 = logit_sb[:].rearrange("p (c o) -> p c o", o=OUT)
        NW = 8
        q = t_steps // NW
        for k in range(NW):
            lo, hi = k * q, (k + 1) * q
            nc.scalar.activation(
                logit_sb[:, lo * OUT : hi * OUT],
                logit_sb[:, lo * OUT : hi * OUT], AF.Exp,
            )
            nc.vector.reduce_sum(
                den_sb[:, lo:hi], p3[:, lo:hi, :], axis=mybir.AxisListType.X
            )
            nc.vector.reciprocal(den_sb[:, lo:hi], den_sb[:, lo:hi])
            rec_b = den_sb[:, lo:hi].unsqueeze(2).broadcast_to([BL, q, OUT])
            nc.vector.tensor_mul(p3[:, lo:hi, :], p3[:, lo:hi, :], rec_b)
            nc.sync.dma_start(d_out.ap()[:, lo:hi, :], p3[:, lo:hi, :])

    if not for_sim:
        _split_overloaded_waits(nc, mybir)
    _BUILD_CACHE[key] = nc
    return nc


def _host_prep(inputs, c0, W_ih, W_hh, b_ih, b_hh, W_fc, b_fc, emb, t_steps):
    import ml_dtypes

    bf16 = ml_dtypes.bfloat16
    inputs = np.asarray(inputs)
    table_f = emb @ W_ih.T + (b_ih + b_hh)  # [7, 512]
    w_f = W_hh.T.copy()  # [128, 512]
    if int(os.environ.get("V3_ACT", "2")) == 2:
        table_f[:, 2 * HID : 3 * HID] *= 2.0
        w_f[:, 2 * HID : 3 * HID] *= 2.0
    table = table_f.astype(bf16)
    w = np.ascontiguousarray(w_f.astype(bf16))
    wfc = np.ascontiguousarray(W_fc.T.astype(bf16))  # [128, 6]
    bfcw = np.ascontiguousarray(
        np.tile(b_fc.astype(bf16), FCW).reshape(1, FCW * OUT)
    )
    in_maps = []
    for c in range(NCORES):
        idx = inputs[c * BL : (c + 1) * BL, :t_steps]  # [32, t]
        oh = np.zeros((VOCAB, t_steps * BL), dtype=bf16)
        cols = np.arange(t_steps * BL)
        vals = idx.T.reshape(-1)  # t-major
        oh[vals, cols] = 1.0
        c0T = np.ascontiguousarray(c0[0, c * BL : (c + 1) * BL, :].T.astype(np.float32))
        in_maps.append(
            {
                "onehot": oh,
                "c0T": c0T,
                "w": w,
                "tbl": table,
                "wfc": wfc,
                "bfcw": bfcw,
            }
        )
    return in_maps


def _run(inputs, c0, W_ih, W_hh, b_ih, b_hh, W_fc, b_fc, emb, t_steps=T,
         trace=False):
    from concourse.bass_utils import run_bass_kernel_spmd

    nc = _build_nc(t_steps)
    in_maps = _host_prep(
        inputs, c0, W_ih, W_hh, b_ih, b_hh, W_fc, b_fc, emb, t_steps
    )
    res = run_bass_kernel_spmd(
        nc, in_maps, core_ids=list(range(NCORES)), trace=trace
    )
    out = np.concatenate([res.results[c]["out"] for c in range(NCORES)], axis=0)
    return out, res


def kernel(inputs, c0, W_ih, W_hh, b_ih, b_hh, W_fc, b_fc, emb):
    out, _ = _run(
        np.asarray(inputs), np.asarray(c0), np.asarray(W_ih), np.asarray(W_hh),
        np.asarray(b_ih), np.asarray(b_hh), np.asarray(W_fc), np.asarray(b_fc),
        np.asarray(emb),
    )
    return out



# revision 2
# speedup vs baseline: 1.0460x; 1.0460x over previous
"""Trainium2 Bass kernel v6 for the decoder LSTM (B=256, T=2048, HID=128, OUT=6).

v4/v5 sharded TIME across the 8 cores (burn-in trick: the LSTM state is
contractive, so a zero state 32 steps before a segment converges to ~2e-7).
Each core ran ONE 288-step chain; the per-step serial chain (PE matmuls ->
ACT sigmoid -> DVE cell update -> ACT tanh -> DVE h-mul) left every engine
half idle.

v6 runs TWO interleaved chains per core (16 time-segments of 128 output
steps + 32 burn-in across 8 cores = 160 rounds of 2 steps). While chain A
is in its DVE phase, chain B uses ACT, etc. ACT is the binding engine
(~3.75us of sigmoid+tanh per round); chains alternate emission priority
each round so neither is systematically the straggler. fc-window
evacuation runs on the otherwise-idle GPSIMD. Filler matmuls keep the PE
HAM activity window saturated so matmuls run at 2.4 GHz.
"""

import os
import sys

for _p in ("/opt/trn_rl_repo", "/root/.axon_site/_ro/trn_rl_repo"):
    if os.path.isdir(_p) and _p not in sys.path:
        sys.path.insert(0, _p)

import numpy as np

B, T, VOCAB, EMB, HID, OUT = 256, 2048, 7, 20, 128, 6
NCORES = 8
VK = VOCAB + 1          # vocab + identity pseudo-token for core 0 burn-in
BURN = 32               # burn-in rounds per chain
NCH = 2                 # chains per core
SEG = T // NCORES       # 256 output steps per core
SEG2 = SEG // NCH       # 128 output steps per chain
TT = SEG2 + BURN        # 160 rounds
FCW = 16                # rounds per fc PSUM window ([128, 4*16*6] f32)
OG = 4                  # rounds per one-hot DMA chunk
NFILL = int(os.environ.get("V6_FILL", "3"))
GI, GF, GG, GO = 0, 1, 2, 3  # PyTorch gate order in W_hh rows / table cols


def _split_overloaded_waits(nc, mybir, max_other=1):
    """walrus in this env rejects instructions with more than a couple of sem
    waits (and InstDrain with any). Move excess waits onto same-engine NoOps
    emitted just before; same-engine program order preserves semantics."""
    n_split = 0
    for f in nc.m.functions:
        for blk in f.blocks:
            out = []
            changed = False
            for inst in blk.instructions:
                si = inst.sync_info
                waits = list(si.on_wait) if si is not None and si.on_wait else []
                limit = 0 if isinstance(inst, mybir.InstDrain) else max_other
                if len(waits) > limit:
                    moved = waits if limit == 0 else waits[limit:]
                    keep = [] if limit == 0 else waits[:limit]
                    for i0, w in enumerate(moved):
                        nop = mybir.InstNoOp(
                            name=f"{inst.name}-wsplit{i0}", ins=[], outs=[]
                        )
                        nop.engine = inst.engine
                        nop.sync_info = mybir.SyncInfo(on_wait=[w], on_update=[])
                        out.append(nop)
                        n_split += 1
                    inst.sync_info = mybir.SyncInfo(
                        on_wait=keep,
                        on_update=list(si.on_update) if si.on_update else [],
                    )
                    changed = True
                out.append(inst)
            if changed:
                blk.instructions = out
    return n_split


def _patch_tile_drain():
    import concourse.tile as tile
    from concourse.vector_clock import ScopedClock, VectorClock

    def _drain_and_barrier_split(self, tick_clock, wait_clock):
        gc = tick_clock.global_clock
        n = len(gc)
        for j in range(n):
            if gc[j] <= 0:
                continue
            vec = [0] * n
            vec[j] = gc[j]
            nop = self.nc.sync.nop(nofuse=True, hint=f"drain_split_{j}")
            wait_clock.add_sem_waits(nop.ins, ScopedClock({None: VectorClock(vec)}))
        self.nc.sync.drain()
        self.nc.all_engine_barrier()
        assert self.sems is not None
        popped = self.nc._tile_sem_poison_stack.pop()
        assert popped is self._sem_poison
        self.nc.clear_and_free_semaphores(list(self.sems.allocated().values()))
        self.nc.all_engine_barrier()

    tile.TileContext._drain_and_barrier = _drain_and_barrier_split


_BUILD_CACHE = {}


def _build_nc(for_sim=False):
    key = (TT, for_sim, NFILL)
    if key in _BUILD_CACHE:
        return _BUILD_CACHE[key]
    import concourse.bass as bass
    import concourse.mybir as mybir
    import concourse.tile as tile

    _patch_tile_drain()

    f32 = mybir.dt.float32
    bf16 = mybir.dt.bfloat16
    AF = mybir.ActivationFunctionType
    ALU = mybir.AluOpType

    nc = bass.Bass("TRN2", target_bir_lowering=False, debug=False)
    d_oh = nc.dram_tensor("onehot", [VK, TT * NCH * B], bf16, kind="ExternalInput")
    d_c0 = nc.dram_tensor("c0T", [HID, NCH * B], bf16, kind="ExternalInput")
    d_w = nc.dram_tensor("w", [HID, 4 * HID], bf16, kind="ExternalInput")
    d_tbl = nc.dram_tensor("tbl", [VK, 4 * HID], bf16, kind="ExternalInput")
    d_wfc = nc.dram_tensor("wfc", [HID, OUT], bf16, kind="ExternalInput")
    d_bfcw = nc.dram_tensor("bfcw", [1, 4 * FCW * OUT], bf16, kind="ExternalInput")
    d_out = nc.dram_tensor("out", [HID, NCH * SEG2 * NCH, OUT], f32,
                           kind="ExternalOutput")

    NWIN = SEG2 // FCW  # 8 fc windows; softmax windows align 1:1

    with tile.TileContext(nc) as tc, tc.tile_pool(name="const", bufs=1) as constp:
        w_sb = constp.tile([HID, 4 * HID], bf16, name="w_sb")
        tbl_sb = constp.tile([VK, 4 * HID], bf16, name="tbl_sb")
        wfc_sb = constp.tile([HID, OUT], bf16, name="wfc_sb")
        bfcw_sb = constp.tile([1, 4 * FCW * OUT], bf16, name="bfcw_sb")
        ones_sb = constp.tile([1, HID], bf16, name="ones_sb")
        cst2 = constp.tile([HID, NCH * B], bf16, name="cst2")
        h0_sb = constp.tile([HID, B], bf16, name="h0_sb")
        scr = constp.tile([HID, B], bf16, name="scr")
        fillsrc = constp.tile([HID, 2 * B], bf16, name="fillsrc")
        logit_sb = constp.tile([HID, NCH * SEG2 * NCH * OUT], f32, name="logit_sb")
        den_sb = constp.tile([HID, NCH * SEG2 * NCH], f32, name="den_sb")

        nc.sync.dma_start(w_sb[:], d_w.ap())
        nc.sync.dma_start(tbl_sb[:], d_tbl.ap())
        nc.sync.dma_start(wfc_sb[:], d_wfc.ap())
        nc.sync.dma_start(bfcw_sb[:], d_bfcw.ap())
        nc.sync.dma_start(cst2[:], d_c0.ap())
        nc.vector.memset(h0_sb[:], 0.0)
        nc.vector.memset(ones_sb[:], 1.0)
        nc.vector.memset(fillsrc[:], 0.0)
        # Pin the sigmoid_and_others table (contains tanh too) before the loop.
        nc.scalar.activation(scr[:], h0_sb[:], AF.Sigmoid)

        cst = [cst2[:, c * B : (c + 1) * B] for c in range(NCH)]

        with (
            tc.tile_pool(name="ohp", bufs=3) as ohp,
            tc.tile_pool(name="gatep", bufs=1, space="PSUM") as gatep,
            tc.tile_pool(name="fcp", bufs=1, space="PSUM") as fcp,
            tc.tile_pool(name="fillp", bufs=1, space="PSUM") as fillp,
            tc.tile_pool(name="ringp", bufs=3) as ringp,
            tc.tile_pool(name="workp", bufs=2) as workp,
        ):
            oh_tiles = [None] * (TT // OG + 1)

            def fetch_oh(chunk):
                ohc = ohp.tile([VK, OG * NCH * B], bf16, tag="oh")
                nc.sync.dma_start(
                    ohc[:],
                    d_oh.ap()[:, chunk * OG * NCH * B : (chunk + 1) * OG * NCH * B],
                )
                oh_tiles[chunk] = ohc

            fetch_oh(0)

            def alloc_pair(r, c):
                """Allocate chain c's round-r gate PSUM pair and prefill from
                the one-hot block (4 matmuls, K=VK, N=256)."""
                psA = gatep.tile([128, 2 * B], f32, tag=f"psA{c}")
                psB = gatep.tile([128, 2 * B], f32, tag=f"psB{c}")
                oh = oh_tiles[r // OG]
                col = ((r % OG) * NCH + c) * B
                for j, q in enumerate((GG, GI, GF, GO)):
                    ps = psA if j < 2 else psB
                    b0 = (j % 2) * B
                    nc.tensor.matmul(
                        ps[:, b0 : b0 + B],
                        tbl_sb[:, q * HID : (q + 1) * HID],
                        oh[:, col : col + B],
                        start=(j % 2 == 0),
                        stop=False,
                        skip_group_check=True,
                    )
                return (psA, psB)

            fill_ps = fillp.tile([128, 2 * B], f32, name="fill_ps")
            cur = [alloc_pair(0, 0), alloc_pair(0, 1)]
            nxt = [None, None]
            fcw_box = [None]
            pending_fc = [None, None]
            h_prev = [h0_sb[:], h0_sb[:]]

            def fc_round(t, order):
                """fc for output step t (both chains). Shared window bank
                [128, (chain, half, FCW, OUT)] f32; evacuated by GPSIMD."""
                w0 = t % FCW
                if w0 == 0:
                    fcw_box[0] = fcp.tile([HID, 4 * FCW * OUT], f32, tag="fcw", name="fcw")
                    nc.tensor.matmul(
                        fcw_box[0][:], ones_sb[:], bfcw_sb[:],
                        start=True, stop=False, skip_group_check=True,
                    )
                fcw = fcw_box[0]
                last_c = order[-1]
                for c in order:
                    for hf in range(2):
                        o0 = (((c * 2) + hf) * FCW + w0) * OUT
                        nc.tensor.matmul(
                            fcw[:, o0 : o0 + OUT],
                            pending_fc[c][:, hf * HID : (hf + 1) * HID],
                            wfc_sb[:],
                            start=False,
                            stop=(w0 == FCW - 1 and hf == 1 and c == last_c),
                            skip_group_check=True,
                        )
                    pending_fc[c] = None
                if w0 == FCW - 1:
                    win = t // FCW
                    nc.scalar.copy(
                        logit_sb[:, win * 4 * FCW * OUT : (win + 1) * 4 * FCW * OUT],
                        fcw[:],
                    )

            tanh_early = os.environ.get("V6_TANH_EARLY", "1") == "1"
            for r in range(TT):
                order = (0, 1) if r % 2 == 0 else (1, 0)
                a, c_b = order
                # --- PE: gate matmuls (critical), then fc, fillers
                for c in order:
                    psA, psB = cur[c]
                    for j, q in enumerate((GG, GI, GF, GO)):
                        ps = psA if j < 2 else psB
                        b0 = (j % 2) * B
                        nc.tensor.matmul(
                            ps[:, b0 : b0 + B],
                            w_sb[:, q * HID : (q + 1) * HID],
                            h_prev[c], start=False, stop=(j % 2 == 1),
                            skip_group_check=True,
                        )
                if pending_fc[0] is not None:
                    fc_round(r - 1 - BURN, order)
                for _ in range(NFILL):
                    nc.tensor.matmul(
                        fill_ps[:, 0:B], w_sb[:, 0:HID], scr[:],
                        start=True, stop=True, skip_group_check=True,
                    )
                # --- ACT sigmoids + DVE cell updates, interleaved so the
                # FIFO queues drain in readiness order. a = lead chain.
                sgi = [None, None]
                sfo = [None, None]
                tg = [None, None]
                ig = [None, None]
                mmb = [None, None]
                tcl = [None, None]
                hsl = [None, None]

                def act_sgi(c):
                    sgi[c] = workp.tile([HID, 2 * B], bf16, tag=f"sgi{c}", name=f"sgi{c}")
                    nc.scalar.activation(sgi[c][:], cur[c][0][:], AF.Sigmoid)

                def act_sfo(c):
                    sfo[c] = workp.tile([HID, 2 * B], bf16, tag=f"sfo{c}", name=f"sfo{c}")
                    nc.scalar.activation(sfo[c][:], cur[c][1][:], AF.Sigmoid)

                def dve_head(c):
                    tg[c] = workp.tile([HID, B], bf16, tag=f"tg{c}", name=f"tg{c}")
                    ig[c] = workp.tile([HID, B], bf16, tag=f"ig{c}", name=f"ig{c}")
                    nc.vector.tensor_scalar(
                        tg[c][:], sgi[c][:, 0:B], 2.0, 1.0,
                        op0=ALU.mult, op1=ALU.subtract,
                    )
                    nc.vector.tensor_mul(ig[c][:], tg[c][:], sgi[c][:, B : 2 * B])

                def dve_cell(c):
                    mmb[c] = workp.tile([HID, B], bf16, tag=f"mm{c}", name=f"mm{c}")
                    nc.vector.tensor_mul(mmb[c][:], sfo[c][:, 0:B], cst[c])
                    nc.vector.tensor_add(cst[c], mmb[c][:], ig[c][:])

                def act_tanh(c):
                    tcl[c] = workp.tile([HID, B], bf16, tag=f"tcl{c}", name=f"tcl{c}")
                    nc.scalar.activation(tcl[c][:], cst[c], AF.Tanh)

                def dve_h(c):
                    hsl[c] = ringp.tile([HID, B], bf16, tag=f"h{c}", name=f"h{c}")
                    nc.vector.tensor_mul(
                        hsl[c][:], sfo[c][:, B : 2 * B], tcl[c][:]
                    )

                if tanh_early:
                    act_sgi(a)
                    act_sfo(a)
                    act_sgi(c_b)
                    dve_head(a)
                    dve_cell(a)
                    act_tanh(a)
                    act_sfo(c_b)
                    dve_head(c_b)
                    dve_h(a)
                    dve_cell(c_b)
                    act_tanh(c_b)
                    dve_h(c_b)
                else:
                    act_sgi(a)
                    act_sfo(a)
                    act_sgi(c_b)
                    act_sfo(c_b)
                    dve_head(a)
                    dve_cell(a)
                    act_tanh(a)
                    dve_head(c_b)
                    dve_h(a)
                    dve_cell(c_b)
                    act_tanh(c_b)
                    dve_h(c_b)

                # --- PE: prefill round r+1 (after the sigmoids that free the
                # recycled banks are emitted — bufs=1 WAR ordering)
                if (r + 1) % OG == 0 and (r + 1) // OG < TT // OG:
                    fetch_oh((r + 1) // OG)
                if r + 1 < TT:
                    for c in order:
                        nxt[c] = alloc_pair(r + 1, c)

                for c in (0, 1):
                    h_prev[c] = hsl[c][:]
                    if r >= BURN:
                        pending_fc[c] = hsl[c][:]
                    if r + 1 < TT:
                        cur[c] = nxt[c]
            fc_round(SEG2 - 1, (0, 1))

        # ---- phase 2: softmax over OUT, windowed; layout is
        # [128, (win, chain, half, w, OUT)] and the host fixes the order. ----
        CDIM = NCH * SEG2 * NCH  # 512 (row groups of OUT)
        p3 = logit_sb[:].rearrange("p (c o) -> p c o", o=OUT)
        NW = 8
        q = CDIM // NW
        for k in range(NW):
            lo, hi = k * q, (k + 1) * q
            nc.scalar.activation(
                logit_sb[:, lo * OUT : hi * OUT],
                logit_sb[:, lo * OUT : hi * OUT], AF.Exp,
            )
            nc.vector.reduce_sum(
                den_sb[:, lo:hi], p3[:, lo:hi, :], axis=mybir.AxisListType.X
            )
            nc.vector.reciprocal(den_sb[:, lo:hi], den_sb[:, lo:hi])
            rec_b = den_sb[:, lo:hi].unsqueeze(2).broadcast_to([HID, q, OUT])
            nc.vector.tensor_mul(p3[:, lo:hi, :], p3[:, lo:hi, :], rec_b)
            nc.sync.dma_start(d_out.ap()[:, lo:hi, :], p3[:, lo:hi, :])

    if not for_sim:
        _split_overloaded_waits(nc, mybir)
    _BUILD_CACHE[key] = nc
    return nc


def _host_prep(inputs, c0, W_ih, W_hh, b_ih, b_hh, W_fc, b_fc, emb):
    import ml_dtypes

    bf16 = ml_dtypes.bfloat16
    inputs = np.asarray(inputs)
    table_f = np.zeros((VK, 4 * HID), np.float32)
    table_f[:VOCAB] = emb @ W_ih.T + (b_ih + b_hh)
    table_f[VOCAB, GI * HID : (GI + 1) * HID] = -30.0
    table_f[VOCAB, GF * HID : (GF + 1) * HID] = 30.0
    table_f[VOCAB, GO * HID : (GO + 1) * HID] = -30.0
    w_f = W_hh.T.copy()
    table_f[:, GG * HID : (GG + 1) * HID] *= 2.0
    w_f[:, GG * HID : (GG + 1) * HID] *= 2.0
    table = table_f.astype(bf16)
    w = np.ascontiguousarray(w_f.astype(bf16))
    wfc = np.ascontiguousarray(W_fc.T.astype(bf16))
    bfcw = np.ascontiguousarray(
        np.tile(b_fc.astype(bf16), 4 * FCW).reshape(1, 4 * FCW * OUT)
    )
    c0T = np.ascontiguousarray(c0[0].T.astype(bf16))
    in_maps = []
    for core in range(NCORES):
        s = core * SEG
        toks = []
        for c in range(NCH):
            st = s + c * SEG2
            if core == 0 and c == 0:
                tok = np.concatenate(
                    [np.full((B, BURN), VOCAB, np.int64), inputs[:, :SEG2]],
                    axis=1,
                )
            else:
                tok = inputs[:, st - BURN : st + SEG2]
            toks.append(tok.T)  # [TT, B]
        tok_all = np.stack(toks, axis=1)  # [TT, NCH, B]
        oh = np.zeros((VK, TT * NCH * B), dtype=bf16)
        cols = np.arange(TT * NCH * B)
        oh[tok_all.reshape(-1), cols] = 1.0
        cc = np.zeros((HID, NCH * B), dtype=bf16)
        if core == 0:
            cc[:, 0:B] = c0T
        in_maps.append(
            {
                "onehot": oh,
                "c0T": cc,
                "w": w,
                "tbl": table,
                "wfc": wfc,
                "bfcw": bfcw,
            }
        )
    return in_maps


def _run(inputs, c0, W_ih, W_hh, b_ih, b_hh, W_fc, b_fc, emb, trace=False):
    from concourse.bass_utils import run_bass_kernel_spmd

    nc = _build_nc()
    in_maps = _host_prep(inputs, c0, W_ih, W_hh, b_ih, b_hh, W_fc, b_fc, emb)
    res = run_bass_kernel_spmd(
        nc, in_maps, core_ids=list(range(NCORES)), trace=trace
    )
    NWIN = SEG2 // FCW
    segs = []
    for core in range(NCORES):
        arr = res.results[core]["out"]  # [128, 512, 6]
        arr = arr.reshape(HID, NWIN, NCH, 2, FCW, OUT)
        # batch = hf*128 + p ; t(core-rel) = c*SEG2 + win*FCW + w
        arr = arr.transpose(3, 0, 2, 1, 4, 5).reshape(B, SEG, OUT)
        segs.append(arr)
    out = np.concatenate(segs, axis=1)
    return out, res


def kernel(inputs, c0, W_ih, W_hh, b_ih, b_hh, W_fc, b_fc, emb):
    out, _ = _run(
        np.asarray(inputs), np.asarray(c0), np.asarray(W_ih), np.asarray(W_hh),
        np.asarray(b_ih), np.asarray(b_hh), np.asarray(W_fc), np.asarray(b_fc),
        np.asarray(emb),
    )
    return out


# revision 3
# speedup vs baseline: 1.1545x; 1.1037x over previous
"""Trainium2 Bass kernel v6 for the decoder LSTM (B=256, T=2048, HID=128, OUT=6).

v4/v5 sharded TIME across the 8 cores (burn-in trick: the LSTM state is
contractive, so a zero state 32 steps before a segment converges to ~2e-7).
Each core ran ONE 288-step chain; the per-step serial chain (PE matmuls ->
ACT sigmoid -> DVE cell update -> ACT tanh -> DVE h-mul) left every engine
half idle.

v6 runs TWO interleaved chains per core (16 time-segments of 128 output
steps + 32 burn-in across 8 cores = 160 rounds of 2 steps). While chain A
is in its DVE phase, chain B uses ACT, etc. ACT is the binding engine
(~3.75us of sigmoid+tanh per round); chains alternate emission priority
each round so neither is systematically the straggler. fc-window
evacuation runs on the otherwise-idle GPSIMD. Filler matmuls keep the PE
HAM activity window saturated so matmuls run at 2.4 GHz.
"""

import os
import sys

for _p in ("/opt/trn_rl_repo", "/root/.axon_site/_ro/trn_rl_repo"):
    if os.path.isdir(_p) and _p not in sys.path:
        sys.path.insert(0, _p)

import numpy as np

B, T, VOCAB, EMB, HID, OUT = 256, 2048, 7, 20, 128, 6
NCORES = 8
VK = VOCAB + 1          # vocab + identity pseudo-token for core 0 burn-in
BURN = 16               # burn-in rounds per chain (zero-state handoff error
                        # at 16 steps measured 1.6e-4 -- far below the bf16
                        # chain noise ~1.3e-3 and the 2e-2 tolerance)
NCH = 2                 # chains per core
SEG = T // NCORES       # 256 output steps per core
SEG2 = SEG // NCH       # 128 output steps per chain
TT = SEG2 + BURN        # 160 rounds
FCW = 16                # rounds per fc PSUM window ([128, 4*16*6] f32)
OG = 4                  # rounds per one-hot DMA chunk
NFILL = int(os.environ.get("V6_FILL", "3"))
GI, GF, GG, GO = 0, 1, 2, 3  # PyTorch gate order in W_hh rows / table cols


def _split_overloaded_waits(nc, mybir, max_other=1):
    """walrus in this env rejects instructions with more than a couple of sem
    waits (and InstDrain with any). Move excess waits onto same-engine NoOps
    emitted just before; same-engine program order preserves semantics."""
    n_split = 0
    for f in nc.m.functions:
        for blk in f.blocks:
            out = []
            changed = False
            for inst in blk.instructions:
                si = inst.sync_info
                waits = list(si.on_wait) if si is not None and si.on_wait else []
                limit = 0 if isinstance(inst, mybir.InstDrain) else max_other
                if len(waits) > limit:
                    moved = waits if limit == 0 else waits[limit:]
                    keep = [] if limit == 0 else waits[:limit]
                    for i0, w in enumerate(moved):
                        nop = mybir.InstNoOp(
                            name=f"{inst.name}-wsplit{i0}", ins=[], outs=[]
                        )
                        nop.engine = inst.engine
                        nop.sync_info = mybir.SyncInfo(on_wait=[w], on_update=[])
                        out.append(nop)
                        n_split += 1
                    inst.sync_info = mybir.SyncInfo(
                        on_wait=keep,
                        on_update=list(si.on_update) if si.on_update else [],
                    )
                    changed = True
                out.append(inst)
            if changed:
                blk.instructions = out
    return n_split


def _patch_tile_drain():
    import concourse.tile as tile
    from concourse.vector_clock import ScopedClock, VectorClock

    def _drain_and_barrier_split(self, tick_clock, wait_clock):
        gc = tick_clock.global_clock
        n = len(gc)
        for j in range(n):
            if gc[j] <= 0:
                continue
            vec = [0] * n
            vec[j] = gc[j]
            nop = self.nc.sync.nop(nofuse=True, hint=f"drain_split_{j}")
            wait_clock.add_sem_waits(nop.ins, ScopedClock({None: VectorClock(vec)}))
        self.nc.sync.drain()
        self.nc.all_engine_barrier()
        assert self.sems is not None
        popped = self.nc._tile_sem_poison_stack.pop()
        assert popped is self._sem_poison
        self.nc.clear_and_free_semaphores(list(self.sems.allocated().values()))
        self.nc.all_engine_barrier()

    tile.TileContext._drain_and_barrier = _drain_and_barrier_split


_BUILD_CACHE = {}


def _build_nc(for_sim=False):
    key = (TT, for_sim, NFILL)
    if key in _BUILD_CACHE:
        return _BUILD_CACHE[key]
    import concourse.bass as bass
    import concourse.mybir as mybir
    import concourse.tile as tile

    _patch_tile_drain()

    f32 = mybir.dt.float32
    bf16 = mybir.dt.bfloat16
    AF = mybir.ActivationFunctionType
    ALU = mybir.AluOpType

    nc = bass.Bass("TRN2", target_bir_lowering=False, debug=False)
    d_oh = nc.dram_tensor("onehot", [VK, TT * NCH * B], bf16, kind="ExternalInput")
    d_c0 = nc.dram_tensor("c0T", [HID, NCH * B], bf16, kind="ExternalInput")
    d_w = nc.dram_tensor("w", [HID, 4 * HID], bf16, kind="ExternalInput")
    d_tbl = nc.dram_tensor("tbl", [VK, 4 * HID], bf16, kind="ExternalInput")
    d_wfc = nc.dram_tensor("wfc", [HID, OUT], bf16, kind="ExternalInput")
    d_bfcw = nc.dram_tensor("bfcw", [1, 4 * FCW * OUT], bf16, kind="ExternalInput")
    d_out = nc.dram_tensor("out", [HID, NCH * SEG2 * NCH, OUT], f32,
                           kind="ExternalOutput")

    NWIN = SEG2 // FCW  # 8 fc windows; softmax windows align 1:1

    with tile.TileContext(nc) as tc, tc.tile_pool(name="const", bufs=1) as constp:
        w_sb = constp.tile([HID, 4 * HID], bf16, name="w_sb")
        tbl_sb = constp.tile([VK, 4 * HID], bf16, name="tbl_sb")
        wfc_sb = constp.tile([HID, OUT], bf16, name="wfc_sb")
        bfcw_sb = constp.tile([1, 4 * FCW * OUT], bf16, name="bfcw_sb")
        ones_sb = constp.tile([1, HID], bf16, name="ones_sb")
        cst2 = constp.tile([HID, NCH * B], bf16, name="cst2")
        h0_sb = constp.tile([HID, B], bf16, name="h0_sb")
        scr = constp.tile([HID, B], bf16, name="scr")
        fillsrc = constp.tile([HID, 2 * B], bf16, name="fillsrc")
        logit_sb = constp.tile([HID, NCH * SEG2 * NCH * OUT], f32, name="logit_sb")
        den_sb = constp.tile([HID, NCH * SEG2 * NCH], f32, name="den_sb")

        nc.sync.dma_start(w_sb[:], d_w.ap())
        nc.sync.dma_start(tbl_sb[:], d_tbl.ap())
        nc.sync.dma_start(wfc_sb[:], d_wfc.ap())
        nc.sync.dma_start(bfcw_sb[:], d_bfcw.ap())
        nc.sync.dma_start(cst2[:], d_c0.ap())
        nc.vector.memset(h0_sb[:], 0.0)
        nc.vector.memset(ones_sb[:], 1.0)
        nc.vector.memset(fillsrc[:], 0.0)
        # Pin the sigmoid_and_others table (contains tanh too) before the loop.
        nc.scalar.activation(scr[:], h0_sb[:], AF.Sigmoid)

        cst = [cst2[:, c * B : (c + 1) * B] for c in range(NCH)]

        with (
            tc.tile_pool(name="ohp", bufs=3) as ohp,
            tc.tile_pool(name="gatep", bufs=1, space="PSUM") as gatep,
            tc.tile_pool(name="fcp", bufs=1, space="PSUM") as fcp,
            tc.tile_pool(name="fillp", bufs=1, space="PSUM") as fillp,
            tc.tile_pool(name="ringp", bufs=3) as ringp,
            tc.tile_pool(name="workp", bufs=2) as workp,
        ):
            oh_tiles = [None] * (TT // OG + 1)

            def fetch_oh(chunk):
                ohc = ohp.tile([VK, OG * NCH * B], bf16, tag="oh")
                nc.sync.dma_start(
                    ohc[:],
                    d_oh.ap()[:, chunk * OG * NCH * B : (chunk + 1) * OG * NCH * B],
                )
                oh_tiles[chunk] = ohc

            fetch_oh(0)

            def alloc_pair(r, c):
                """Allocate chain c's round-r gate PSUM pair and prefill from
                the one-hot block (4 matmuls, K=VK, N=256)."""
                psA = gatep.tile([128, 2 * B], f32, tag=f"psA{c}")
                psB = gatep.tile([128, 2 * B], f32, tag=f"psB{c}")
                oh = oh_tiles[r // OG]
                col = ((r % OG) * NCH + c) * B
                for j, q in enumerate((GG, GI, GF, GO)):
                    ps = psA if j < 2 else psB
                    b0 = (j % 2) * B
                    nc.tensor.matmul(
                        ps[:, b0 : b0 + B],
                        tbl_sb[:, q * HID : (q + 1) * HID],
                        oh[:, col : col + B],
                        start=(j % 2 == 0),
                        stop=False,
                        skip_group_check=True,
                    )
                return (psA, psB)

            fill_ps = fillp.tile([128, 2 * B], f32, name="fill_ps")
            cur = [alloc_pair(0, 0), alloc_pair(0, 1)]
            nxt = [None, None]
            fcw_box = [None]
            pending_fc = [None, None]
            h_prev = [h0_sb[:], h0_sb[:]]

            def fc_round(t, order):
                """fc for output step t (both chains). Shared window bank
                [128, (chain, half, FCW, OUT)] f32; evacuated by GPSIMD."""
                w0 = t % FCW
                if w0 == 0:
                    fcw_box[0] = fcp.tile([HID, 4 * FCW * OUT], f32, tag="fcw", name="fcw")
                    nc.tensor.matmul(
                        fcw_box[0][:], ones_sb[:], bfcw_sb[:],
                        start=True, stop=False, skip_group_check=True,
                    )
                fcw = fcw_box[0]
                last_c = order[-1]
                for c in order:
                    for hf in range(2):
                        o0 = (((c * 2) + hf) * FCW + w0) * OUT
                        nc.tensor.matmul(
                            fcw[:, o0 : o0 + OUT],
                            pending_fc[c][:, hf * HID : (hf + 1) * HID],
                            wfc_sb[:],
                            start=False,
                            stop=(w0 == FCW - 1 and hf == 1 and c == last_c),
                            skip_group_check=True,
                        )
                    pending_fc[c] = None
                if w0 == FCW - 1:
                    win = t // FCW
                    nc.scalar.copy(
                        logit_sb[:, win * 4 * FCW * OUT : (win + 1) * 4 * FCW * OUT],
                        fcw[:],
                    )

            tanh_early = os.environ.get("V6_TANH_EARLY", "1") == "1"
            for r in range(TT):
                order = (0, 1) if r % 2 == 0 else (1, 0)
                a, c_b = order
                # --- PE: gate matmuls (critical), then fc, fillers
                for c in order:
                    psA, psB = cur[c]
                    for j, q in enumerate((GG, GI, GF, GO)):
                        ps = psA if j < 2 else psB
                        b0 = (j % 2) * B
                        nc.tensor.matmul(
                            ps[:, b0 : b0 + B],
                            w_sb[:, q * HID : (q + 1) * HID],
                            h_prev[c], start=False, stop=(j % 2 == 1),
                            skip_group_check=True,
                        )
                if pending_fc[0] is not None:
                    fc_round(r - 1 - BURN, order)
                for _ in range(NFILL):
                    nc.tensor.matmul(
                        fill_ps[:, 0:B], w_sb[:, 0:HID], scr[:],
                        start=True, stop=True, skip_group_check=True,
                    )
                # --- ACT sigmoids + DVE cell updates, interleaved so the
                # FIFO queues drain in readiness order. a = lead chain.
                sgi = [None, None]
                sfo = [None, None]
                tg = [None, None]
                ig = [None, None]
                mmb = [None, None]
                tcl = [None, None]
                hsl = [None, None]

                def act_sgi(c):
                    sgi[c] = workp.tile([HID, 2 * B], bf16, tag=f"sgi{c}", name=f"sgi{c}")
                    nc.scalar.activation(sgi[c][:], cur[c][0][:], AF.Sigmoid)

                def act_sfo(c):
                    sfo[c] = workp.tile([HID, 2 * B], bf16, tag=f"sfo{c}", name=f"sfo{c}")
                    nc.scalar.activation(sfo[c][:], cur[c][1][:], AF.Sigmoid)

                def dve_head(c):
                    tg[c] = workp.tile([HID, B], bf16, tag=f"tg{c}", name=f"tg{c}")
                    ig[c] = workp.tile([HID, B], bf16, tag=f"ig{c}", name=f"ig{c}")
                    nc.vector.tensor_scalar(
                        tg[c][:], sgi[c][:, 0:B], 2.0, 1.0,
                        op0=ALU.mult, op1=ALU.subtract,
                    )
                    nc.vector.tensor_mul(ig[c][:], tg[c][:], sgi[c][:, B : 2 * B])

                def dve_cell(c):
                    mmb[c] = workp.tile([HID, B], bf16, tag=f"mm{c}", name=f"mm{c}")
                    nc.vector.tensor_mul(mmb[c][:], sfo[c][:, 0:B], cst[c])
                    nc.vector.tensor_add(cst[c], mmb[c][:], ig[c][:])

                def act_tanh(c):
                    tcl[c] = workp.tile([HID, B], bf16, tag=f"tcl{c}", name=f"tcl{c}")
                    nc.scalar.activation(tcl[c][:], cst[c], AF.Tanh)

                def dve_h(c):
                    hsl[c] = ringp.tile([HID, B], bf16, tag=f"h{c}", name=f"h{c}")
                    nc.vector.tensor_mul(
                        hsl[c][:], sfo[c][:, B : 2 * B], tcl[c][:]
                    )

                if tanh_early:
                    act_sgi(a)
                    act_sfo(a)
                    act_sgi(c_b)
                    dve_head(a)
                    dve_cell(a)
                    act_tanh(a)
                    act_sfo(c_b)
                    dve_head(c_b)
                    dve_h(a)
                    dve_cell(c_b)
                    act_tanh(c_b)
                    dve_h(c_b)
                else:
                    act_sgi(a)
                    act_sfo(a)
                    act_sgi(c_b)
                    act_sfo(c_b)
                    dve_head(a)
                    dve_cell(a)
                    act_tanh(a)
                    dve_head(c_b)
                    dve_h(a)
                    dve_cell(c_b)
                    act_tanh(c_b)
                    dve_h(c_b)

                # --- PE: prefill round r+1 (after the sigmoids that free the
                # recycled banks are emitted — bufs=1 WAR ordering)
                if (r + 1) % OG == 0 and (r + 1) // OG < TT // OG:
                    fetch_oh((r + 1) // OG)
                if r + 1 < TT:
                    for c in order:
                        nxt[c] = alloc_pair(r + 1, c)

                for c in (0, 1):
                    h_prev[c] = hsl[c][:]
                    if r >= BURN:
                        pending_fc[c] = hsl[c][:]
                    if r + 1 < TT:
                        cur[c] = nxt[c]
            fc_round(SEG2 - 1, (0, 1))

        # ---- phase 2: softmax over OUT, windowed; layout is
        # [128, (win, chain, half, w, OUT)] and the host fixes the order. ----
        CDIM = NCH * SEG2 * NCH  # 512 (row groups of OUT)
        p3 = logit_sb[:].rearrange("p (c o) -> p c o", o=OUT)
        NW = 8
        q = CDIM // NW
        for k in range(NW):
            lo, hi = k * q, (k + 1) * q
            nc.scalar.activation(
                logit_sb[:, lo * OUT : hi * OUT],
                logit_sb[:, lo * OUT : hi * OUT], AF.Exp,
            )
            nc.vector.reduce_sum(
                den_sb[:, lo:hi], p3[:, lo:hi, :], axis=mybir.AxisListType.X
            )
            nc.vector.reciprocal(den_sb[:, lo:hi], den_sb[:, lo:hi])
            rec_b = den_sb[:, lo:hi].unsqueeze(2).broadcast_to([HID, q, OUT])
            nc.vector.tensor_mul(p3[:, lo:hi, :], p3[:, lo:hi, :], rec_b)
            nc.sync.dma_start(d_out.ap()[:, lo:hi, :], p3[:, lo:hi, :])

    if not for_sim:
        _split_overloaded_waits(nc, mybir)
    _BUILD_CACHE[key] = nc
    return nc


def _host_prep(inputs, c0, W_ih, W_hh, b_ih, b_hh, W_fc, b_fc, emb):
    import ml_dtypes

    bf16 = ml_dtypes.bfloat16
    inputs = np.asarray(inputs)
    table_f = np.zeros((VK, 4 * HID), np.float32)
    table_f[:VOCAB] = emb @ W_ih.T + (b_ih + b_hh)
    table_f[VOCAB, GI * HID : (GI + 1) * HID] = -30.0
    table_f[VOCAB, GF * HID : (GF + 1) * HID] = 30.0
    table_f[VOCAB, GO * HID : (GO + 1) * HID] = -30.0
    w_f = W_hh.T.copy()
    table_f[:, GG * HID : (GG + 1) * HID] *= 2.0
    w_f[:, GG * HID : (GG + 1) * HID] *= 2.0
    table = table_f.astype(bf16)
    w = np.ascontiguousarray(w_f.astype(bf16))
    wfc = np.ascontiguousarray(W_fc.T.astype(bf16))
    bfcw = np.ascontiguousarray(
        np.tile(b_fc.astype(bf16), 4 * FCW).reshape(1, 4 * FCW * OUT)
    )
    c0T = np.ascontiguousarray(c0[0].T.astype(bf16))
    in_maps = []
    for core in range(NCORES):
        s = core * SEG
        toks = []
        for c in range(NCH):
            st = s + c * SEG2
            if core == 0 and c == 0:
                tok = np.concatenate(
                    [np.full((B, BURN), VOCAB, np.int64), inputs[:, :SEG2]],
                    axis=1,
                )
            else:
                tok = inputs[:, st - BURN : st + SEG2]
            toks.append(tok.T)  # [TT, B]
        tok_all = np.stack(toks, axis=1)  # [TT, NCH, B]
        oh = np.zeros((VK, TT * NCH * B), dtype=bf16)
        cols = np.arange(TT * NCH * B)
        oh[tok_all.reshape(-1), cols] = 1.0
        cc = np.zeros((HID, NCH * B), dtype=bf16)
        if core == 0:
            cc[:, 0:B] = c0T
        in_maps.append(
            {
                "onehot": oh,
                "c0T": cc,
                "w": w,
                "tbl": table,
                "wfc": wfc,
                "bfcw": bfcw,
            }
        )
    return in_maps


def _run(inputs, c0, W_ih, W_hh, b_ih, b_hh, W_fc, b_fc, emb, trace=False):
    from concourse.bass_utils import run_bass_kernel_spmd

    nc = _build_nc()
    in_maps = _host_prep(inputs, c0, W_ih, W_hh, b_ih, b_hh, W_fc, b_fc, emb)
    res = run_bass_kernel_spmd(
        nc, in_maps, core_ids=list(range(NCORES)), trace=trace
    )
    NWIN = SEG2 // FCW
    segs = []
    for core in range(NCORES):
        arr = res.results[core]["out"]  # [128, 512, 6]
        arr = arr.reshape(HID, NWIN, NCH, 2, FCW, OUT)
        # batch = hf*128 + p ; t(core-rel) = c*SEG2 + win*FCW + w
        arr = arr.transpose(3, 0, 2, 1, 4, 5).reshape(B, SEG, OUT)
        segs.append(arr)
    out = np.concatenate(segs, axis=1)
    return out, res


def kernel(inputs, c0, W_ih, W_hh, b_ih, b_hh, W_fc, b_fc, emb):
    out, _ = _run(
        np.asarray(inputs), np.asarray(c0), np.asarray(W_ih), np.asarray(W_hh),
        np.asarray(b_ih), np.asarray(b_hh), np.asarray(W_fc), np.asarray(b_fc),
        np.asarray(emb),
    )
    return out


# revision 9
# speedup vs baseline: 1.2115x; 1.0494x over previous
"""Trainium2 Bass kernel v6 for the decoder LSTM (B=256, T=2048, HID=128, OUT=6).

v4/v5 sharded TIME across the 8 cores (burn-in trick: the LSTM state is
contractive, so a zero state 32 steps before a segment converges to ~2e-7).
Each core ran ONE 288-step chain; the per-step serial chain (PE matmuls ->
ACT sigmoid -> DVE cell update -> ACT tanh -> DVE h-mul) left every engine
half idle.

v6 runs TWO interleaved chains per core (16 time-segments of 128 output
steps + 16 burn-in across 8 cores = 144 rounds of 2 steps; measured
678 us on trn2, vs 4203 us for the batch-parallel v3). While chain A
is in its DVE phase, chain B uses ACT, etc. ACT is the binding engine
(~3.75us of sigmoid+tanh per round); chains alternate emission priority
each round so neither is systematically the straggler. Filler matmuls
keep the PE HAM activity window saturated so matmuls run at 2.4 GHz.
"""

import os
import sys

for _p in ("/opt/trn_rl_repo", "/root/.axon_site/_ro/trn_rl_repo"):
    if os.path.isdir(_p) and _p not in sys.path:
        sys.path.insert(0, _p)

import numpy as np

B, T, VOCAB, EMB, HID, OUT = 256, 2048, 7, 20, 128, 6
NCORES = 8
VK = VOCAB + 1          # vocab + identity pseudo-token for core 0 burn-in
BURN = 16               # burn-in rounds per chain (zero-state handoff error
                        # at 16 steps measured 1.6e-4 -- far below the bf16
                        # chain noise ~1.3e-3 and the 2e-2 tolerance)
NCH = 2                 # chains per core
SEG = T // NCORES       # 256 output steps per core
SEG2 = SEG // NCH       # 128 output steps per chain
TT = SEG2 + BURN        # 144 rounds
FCW = 16                # rounds per fc PSUM window ([128, 4*16*6] f32)
OG = 4                  # rounds per one-hot DMA chunk
NFILL = int(os.environ.get("V6_FILL", "3"))
GI, GF, GG, GO = 0, 1, 2, 3  # PyTorch gate order in W_hh rows / table cols


def _split_overloaded_waits(nc, mybir, max_other=1):
    """walrus in this env rejects instructions with more than a couple of sem
    waits (and InstDrain with any). Move excess waits onto same-engine NoOps
    emitted just before; same-engine program order preserves semantics."""
    n_split = 0
    for f in nc.m.functions:
        for blk in f.blocks:
            out = []
            changed = False
            for inst in blk.instructions:
                si = inst.sync_info
                waits = list(si.on_wait) if si is not None and si.on_wait else []
                limit = 0 if isinstance(inst, mybir.InstDrain) else max_other
                if len(waits) > limit:
                    moved = waits if limit == 0 else waits[limit:]
                    keep = [] if limit == 0 else waits[:limit]
                    for i0, w in enumerate(moved):
                        nop = mybir.InstNoOp(
                            name=f"{inst.name}-wsplit{i0}", ins=[], outs=[]
                        )
                        nop.engine = inst.engine
                        nop.sync_info = mybir.SyncInfo(on_wait=[w], on_update=[])
                        out.append(nop)
                        n_split += 1
                    inst.sync_info = mybir.SyncInfo(
                        on_wait=keep,
                        on_update=list(si.on_update) if si.on_update else [],
                    )
                    changed = True
                out.append(inst)
            if changed:
                blk.instructions = out
    return n_split


def _patch_tile_drain():
    import concourse.tile as tile
    from concourse.vector_clock import ScopedClock, VectorClock

    def _drain_and_barrier_split(self, tick_clock, wait_clock):
        gc = tick_clock.global_clock
        n = len(gc)
        for j in range(n):
            if gc[j] <= 0:
                continue
            vec = [0] * n
            vec[j] = gc[j]
            nop = self.nc.sync.nop(nofuse=True, hint=f"drain_split_{j}")
            wait_clock.add_sem_waits(nop.ins, ScopedClock({None: VectorClock(vec)}))
        self.nc.sync.drain()
        self.nc.all_engine_barrier()
        assert self.sems is not None
        popped = self.nc._tile_sem_poison_stack.pop()
        assert popped is self._sem_poison
        self.nc.clear_and_free_semaphores(list(self.sems.allocated().values()))
        self.nc.all_engine_barrier()

    tile.TileContext._drain_and_barrier = _drain_and_barrier_split


_BUILD_CACHE = {}


def _build_nc(for_sim=False):
    key = (TT, for_sim, NFILL)
    if key in _BUILD_CACHE:
        return _BUILD_CACHE[key]
    import concourse.bass as bass
    import concourse.mybir as mybir
    import concourse.tile as tile

    _patch_tile_drain()

    f32 = mybir.dt.float32
    bf16 = mybir.dt.bfloat16
    AF = mybir.ActivationFunctionType
    ALU = mybir.AluOpType

    nc = bass.Bass("TRN2", target_bir_lowering=False, debug=False)
    d_oh = nc.dram_tensor("onehot", [VK, TT * NCH * B], bf16, kind="ExternalInput")
    d_c0 = nc.dram_tensor("c0T", [HID, NCH * B], bf16, kind="ExternalInput")
    d_w = nc.dram_tensor("w", [HID, 4 * HID], bf16, kind="ExternalInput")
    d_tbl = nc.dram_tensor("tbl", [VK, 4 * HID], bf16, kind="ExternalInput")
    d_wfc = nc.dram_tensor("wfc", [HID, OUT], bf16, kind="ExternalInput")
    d_bfcw = nc.dram_tensor("bfcw", [1, 4 * FCW * OUT], bf16, kind="ExternalInput")
    d_out = nc.dram_tensor("out", [HID, NCH * SEG2 * NCH, OUT], f32,
                           kind="ExternalOutput")

    NWIN = SEG2 // FCW  # 8 fc windows; softmax windows align 1:1

    with tile.TileContext(nc) as tc, tc.tile_pool(name="const", bufs=1) as constp:
        w_sb = constp.tile([HID, 4 * HID], bf16, name="w_sb")
        tbl_sb = constp.tile([VK, 4 * HID], bf16, name="tbl_sb")
        wfc_sb = constp.tile([HID, OUT], bf16, name="wfc_sb")
        bfcw_sb = constp.tile([1, 4 * FCW * OUT], bf16, name="bfcw_sb")
        ones_sb = constp.tile([1, HID], bf16, name="ones_sb")
        cst2 = constp.tile([HID, NCH * B], bf16, name="cst2")
        h0_sb = constp.tile([HID, B], bf16, name="h0_sb")
        scr = constp.tile([HID, B], bf16, name="scr")
        fillsrc = constp.tile([HID, 2 * B], bf16, name="fillsrc")
        logit_sb = constp.tile([HID, NCH * SEG2 * NCH * OUT], f32, name="logit_sb")
        den_sb = constp.tile([HID, NCH * SEG2 * NCH], f32, name="den_sb")

        nc.sync.dma_start(w_sb[:], d_w.ap())
        nc.sync.dma_start(tbl_sb[:], d_tbl.ap())
        nc.sync.dma_start(wfc_sb[:], d_wfc.ap())
        nc.sync.dma_start(bfcw_sb[:], d_bfcw.ap())
        nc.sync.dma_start(cst2[:], d_c0.ap())
        nc.vector.memset(h0_sb[:], 0.0)
        nc.vector.memset(ones_sb[:], 1.0)
        nc.vector.memset(fillsrc[:], 0.0)
        # Pin the sigmoid_and_others table (contains tanh too) before the loop.
        nc.scalar.activation(scr[:], h0_sb[:], AF.Sigmoid)

        cst = [cst2[:, c * B : (c + 1) * B] for c in range(NCH)]

        with (
            tc.tile_pool(name="ohp", bufs=3) as ohp,
            tc.tile_pool(name="gatep", bufs=1, space="PSUM") as gatep,
            tc.tile_pool(name="fcp", bufs=1, space="PSUM") as fcp,
            tc.tile_pool(name="fillp", bufs=1, space="PSUM") as fillp,
            tc.tile_pool(name="ringp", bufs=3) as ringp,
            tc.tile_pool(name="workp", bufs=2) as workp,
        ):
            oh_tiles = [None] * (TT // OG + 1)

            def fetch_oh(chunk):
                ohc = ohp.tile([VK, OG * NCH * B], bf16, tag="oh")
                nc.sync.dma_start(
                    ohc[:],
                    d_oh.ap()[:, chunk * OG * NCH * B : (chunk + 1) * OG * NCH * B],
                )
                oh_tiles[chunk] = ohc

            fetch_oh(0)

            mergesig = os.environ.get("V6_MERGESIG", "0") == "1"

            def alloc_pair(r, c):
                """Allocate chain c's round-r gate PSUM (pair of banks, or one
                2-bank tile in mergesig mode) and prefill from the one-hot
                block (4 matmuls, K=VK, N=256). Quarter order [2g|i|f|o]."""
                if mergesig:
                    psAB = gatep.tile([128, 4 * B], f32, tag=f"psAB{c}")
                    quarters = [psAB[:, j * B : (j + 1) * B] for j in range(4)]
                else:
                    psA = gatep.tile([128, 2 * B], f32, tag=f"psA{c}")
                    psB = gatep.tile([128, 2 * B], f32, tag=f"psB{c}")
                    quarters = [
                        psA[:, 0:B], psA[:, B : 2 * B],
                        psB[:, 0:B], psB[:, B : 2 * B],
                    ]
                oh = oh_tiles[r // OG]
                col = ((r % OG) * NCH + c) * B
                for j, q in enumerate((GG, GI, GF, GO)):
                    nc.tensor.matmul(
                        quarters[j],
                        tbl_sb[:, q * HID : (q + 1) * HID],
                        oh[:, col : col + B],
                        start=(j % 2 == 0),
                        stop=False,
                        skip_group_check=True,
                    )
                if mergesig:
                    return (psAB, quarters)
                return (psA, psB, quarters)

            fill_ps = fillp.tile([128, 2 * B], f32, name="fill_ps")
            cur = [alloc_pair(0, 0), alloc_pair(0, 1)]
            nxt = [None, None]
            fcw_box = [None]
            pending_fc = [None, None]
            h_prev = [h0_sb[:], h0_sb[:]]

            def fc_round(t, order):
                """fc for output step t (both chains). Shared window bank
                [128, (chain, half, FCW, OUT)] f32; evacuated by GPSIMD."""
                w0 = t % FCW
                if w0 == 0:
                    fcw_box[0] = fcp.tile([HID, 4 * FCW * OUT], f32, tag="fcw", name="fcw")
                    nc.tensor.matmul(
                        fcw_box[0][:], ones_sb[:], bfcw_sb[:],
                        start=True, stop=False, skip_group_check=True,
                    )
                fcw = fcw_box[0]
                last_c = order[-1]
                for c in order:
                    for hf in range(2):
                        o0 = (((c * 2) + hf) * FCW + w0) * OUT
                        nc.tensor.matmul(
                            fcw[:, o0 : o0 + OUT],
                            pending_fc[c][:, hf * HID : (hf + 1) * HID],
                            wfc_sb[:],
                            start=False,
                            stop=(w0 == FCW - 1 and hf == 1 and c == last_c),
                            skip_group_check=True,
                        )
                    pending_fc[c] = None
                if w0 == FCW - 1:
                    win = t // FCW
                    nc.scalar.copy(
                        logit_sb[:, win * 4 * FCW * OUT : (win + 1) * 4 * FCW * OUT],
                        fcw[:],
                    )

            tanh_early = os.environ.get("V6_TANH_EARLY", "1") == "1"
            for r in range(TT):
                order = (0, 1) if r % 2 == 0 else (1, 0)
                a, c_b = order
                # --- PE: gate matmuls (critical), then fc, fillers
                for c in order:
                    quarters = cur[c][-1]
                    for j, q in enumerate((GG, GI, GF, GO)):
                        nc.tensor.matmul(
                            quarters[j],
                            w_sb[:, q * HID : (q + 1) * HID],
                            h_prev[c], start=False, stop=(j % 2 == 1),
                            skip_group_check=True,
                        )
                if pending_fc[0] is not None:
                    fc_round(r - 1 - BURN, order)
                for _ in range(NFILL):
                    nc.tensor.matmul(
                        fill_ps[:, 0:B], w_sb[:, 0:HID], scr[:],
                        start=True, stop=True, skip_group_check=True,
                    )
                # --- ACT sigmoids + DVE cell updates, interleaved so the
                # FIFO queues drain in readiness order. a = lead chain.
                sgi = [None, None]
                sfo = [None, None]
                tg = [None, None]
                ig = [None, None]
                mmb = [None, None]
                tcl = [None, None]
                hsl = [None, None]

                def act_sgi(c):
                    if mergesig:
                        sgi[c] = workp.tile(
                            [HID, 4 * B], bf16, tag=f"sgi{c}", name=f"sgi{c}"
                        )
                        nc.scalar.activation(sgi[c][:], cur[c][0][:], AF.Sigmoid)
                        sfo[c] = sgi[c]
                    else:
                        sgi[c] = workp.tile(
                            [HID, 2 * B], bf16, tag=f"sgi{c}", name=f"sgi{c}"
                        )
                        nc.scalar.activation(sgi[c][:], cur[c][0][:], AF.Sigmoid)

                def act_sfo(c):
                    if mergesig:
                        return
                    sfo[c] = workp.tile([HID, 2 * B], bf16, tag=f"sfo{c}", name=f"sfo{c}")
                    nc.scalar.activation(sfo[c][:], cur[c][1][:], AF.Sigmoid)

                def dve_head(c):
                    tg[c] = workp.tile([HID, B], bf16, tag=f"tg{c}", name=f"tg{c}")
                    ig[c] = workp.tile([HID, B], bf16, tag=f"ig{c}", name=f"ig{c}")
                    nc.vector.tensor_scalar(
                        tg[c][:], sgi[c][:, 0:B], 2.0, 1.0,
                        op0=ALU.mult, op1=ALU.subtract,
                    )
                    nc.vector.tensor_mul(ig[c][:], tg[c][:], sgi[c][:, B : 2 * B])

                def dve_cell(c):
                    sf_off = 2 * B if mergesig else 0
                    mmb[c] = workp.tile([HID, B], bf16, tag=f"mm{c}", name=f"mm{c}")
                    nc.vector.tensor_mul(
                        mmb[c][:], sfo[c][:, sf_off : sf_off + B], cst[c]
                    )
                    nc.vector.tensor_add(cst[c], mmb[c][:], ig[c][:])

                def act_tanh(c):
                    tcl[c] = workp.tile([HID, B], bf16, tag=f"tcl{c}", name=f"tcl{c}")
                    nc.scalar.activation(tcl[c][:], cst[c], AF.Tanh)

                def dve_h(c):
                    so_off = 3 * B if mergesig else B
                    hsl[c] = ringp.tile([HID, B], bf16, tag=f"h{c}", name=f"h{c}")
                    nc.vector.tensor_mul(
                        hsl[c][:], sfo[c][:, so_off : so_off + B], tcl[c][:]
                    )

                if tanh_early:
                    act_sgi(a)
                    act_sfo(a)
                    act_sgi(c_b)
                    dve_head(a)
                    dve_cell(a)
                    act_tanh(a)
                    act_sfo(c_b)
                    dve_head(c_b)
                    dve_h(a)
                    dve_cell(c_b)
                    act_tanh(c_b)
                    dve_h(c_b)
                else:
                    act_sgi(a)
                    act_sfo(a)
                    act_sgi(c_b)
                    act_sfo(c_b)
                    dve_head(a)
                    dve_cell(a)
                    act_tanh(a)
                    dve_head(c_b)
                    dve_h(a)
                    dve_cell(c_b)
                    act_tanh(c_b)
                    dve_h(c_b)

                # --- PE: prefill round r+1 (after the sigmoids that free the
                # recycled banks are emitted — bufs=1 WAR ordering)
                if (r + 1) % OG == 0 and (r + 1) // OG < TT // OG:
                    fetch_oh((r + 1) // OG)
                if r + 1 < TT:
                    for c in order:
                        nxt[c] = alloc_pair(r + 1, c)

                for c in (0, 1):
                    h_prev[c] = hsl[c][:]
                    if r >= BURN:
                        pending_fc[c] = hsl[c][:]
                    if r + 1 < TT:
                        cur[c] = nxt[c]
            fc_round(SEG2 - 1, (0, 1))

        # ---- phase 2: softmax over OUT, windowed; layout is
        # [128, (win, chain, half, w, OUT)] and the host fixes the order. ----
        CDIM = NCH * SEG2 * NCH  # 512 (row groups of OUT)
        p3 = logit_sb[:].rearrange("p (c o) -> p c o", o=OUT)
        NW = 8
        q = CDIM // NW
        for k in range(NW):
            lo, hi = k * q, (k + 1) * q
            nc.scalar.activation(
                logit_sb[:, lo * OUT : hi * OUT],
                logit_sb[:, lo * OUT : hi * OUT], AF.Exp,
            )
            nc.vector.reduce_sum(
                den_sb[:, lo:hi], p3[:, lo:hi, :], axis=mybir.AxisListType.X
            )
            nc.vector.reciprocal(den_sb[:, lo:hi], den_sb[:, lo:hi])
            rec_b = den_sb[:, lo:hi].unsqueeze(2).broadcast_to([HID, q, OUT])
            nc.vector.tensor_mul(p3[:, lo:hi, :], p3[:, lo:hi, :], rec_b)
            nc.sync.dma_start(d_out.ap()[:, lo:hi, :], p3[:, lo:hi, :])

    if not for_sim:
        _split_overloaded_waits(nc, mybir)
    _BUILD_CACHE[key] = nc
    return nc


def _host_prep(inputs, c0, W_ih, W_hh, b_ih, b_hh, W_fc, b_fc, emb):
    import ml_dtypes

    bf16 = ml_dtypes.bfloat16
    inputs = np.asarray(inputs)
    table_f = np.zeros((VK, 4 * HID), np.float32)
    table_f[:VOCAB] = emb @ W_ih.T + (b_ih + b_hh)
    table_f[VOCAB, GI * HID : (GI + 1) * HID] = -30.0
    table_f[VOCAB, GF * HID : (GF + 1) * HID] = 30.0
    table_f[VOCAB, GO * HID : (GO + 1) * HID] = -30.0
    w_f = W_hh.T.copy()
    table_f[:, GG * HID : (GG + 1) * HID] *= 2.0
    w_f[:, GG * HID : (GG + 1) * HID] *= 2.0
    table = table_f.astype(bf16)
    w = np.ascontiguousarray(w_f.astype(bf16))
    wfc = np.ascontiguousarray(W_fc.T.astype(bf16))
    bfcw = np.ascontiguousarray(
        np.tile(b_fc.astype(bf16), 4 * FCW).reshape(1, 4 * FCW * OUT)
    )
    c0T = np.ascontiguousarray(c0[0].T.astype(bf16))
    in_maps = []
    for core in range(NCORES):
        s = core * SEG
        toks = []
        for c in range(NCH):
            st = s + c * SEG2
            if core == 0 and c == 0:
                tok = np.concatenate(
                    [np.full((B, BURN), VOCAB, np.int64), inputs[:, :SEG2]],
                    axis=1,
                )
            else:
                tok = inputs[:, st - BURN : st + SEG2]
            toks.append(tok.T)  # [TT, B]
        tok_all = np.stack(toks, axis=1)  # [TT, NCH, B]
        oh = np.zeros((VK, TT * NCH * B), dtype=bf16)
        cols = np.arange(TT * NCH * B)
        oh[tok_all.reshape(-1), cols] = 1.0
        cc = np.zeros((HID, NCH * B), dtype=bf16)
        if core == 0:
            cc[:, 0:B] = c0T
        in_maps.append(
            {
                "onehot": oh,
                "c0T": cc,
                "w": w,
                "tbl": table,
                "wfc": wfc,
                "bfcw": bfcw,
            }
        )
    return in_maps


def _run(inputs, c0, W_ih, W_hh, b_ih, b_hh, W_fc, b_fc, emb, trace=False):
    from concourse.bass_utils import run_bass_kernel_spmd

    nc = _build_nc()
    in_maps = _host_prep(inputs, c0, W_ih, W_hh, b_ih, b_hh, W_fc, b_fc, emb)
    res = run_bass_kernel_spmd(
        nc, in_maps, core_ids=list(range(NCORES)), trace=trace
    )
    NWIN = SEG2 // FCW
    segs = []
    for core in range(NCORES):
        arr = res.results[core]["out"]  # [128, 512, 6]
        arr = arr.reshape(HID, NWIN, NCH, 2, FCW, OUT)
        # batch = hf*128 + p ; t(core-rel) = c*SEG2 + win*FCW + w
        arr = arr.transpose(3, 0, 2, 1, 4, 5).reshape(B, SEG, OUT)
        segs.append(arr)
    out = np.concatenate(segs, axis=1)
    return out, res


def kernel(inputs, c0, W_ih, W_hh, b_ih, b_hh, W_fc, b_fc, emb):
    out, _ = _run(
        np.asarray(inputs), np.asarray(c0), np.asarray(W_ih), np.asarray(W_hh),
        np.asarray(b_ih), np.asarray(b_hh), np.asarray(W_fc), np.asarray(b_fc),
        np.asarray(emb),
    )
    return out


# revision 10
# speedup vs baseline: 1.2774x; 1.0544x over previous
"""Trainium2 Bass kernel v6 for the decoder LSTM (B=256, T=2048, HID=128, OUT=6).

v4/v5 sharded TIME across the 8 cores (burn-in trick: the LSTM state is
contractive, so a zero state 32 steps before a segment converges to ~2e-7).
Each core ran ONE 288-step chain; the per-step serial chain (PE matmuls ->
ACT sigmoid -> DVE cell update -> ACT tanh -> DVE h-mul) left every engine
half idle.

v6 runs TWO interleaved chains per core (16 time-segments of 128 output
steps + 16 burn-in across 8 cores = 144 rounds of 2 steps; measured
678 us on trn2, vs 4203 us for the batch-parallel v3). While chain A
is in its DVE phase, chain B uses ACT, etc. ACT is the binding engine
(~3.75us of sigmoid+tanh per round); chains alternate emission priority
each round so neither is systematically the straggler. Filler matmuls
keep the PE HAM activity window saturated so matmuls run at 2.4 GHz.
"""

import os
import sys

for _p in ("/opt/trn_rl_repo", "/root/.axon_site/_ro/trn_rl_repo"):
    if os.path.isdir(_p) and _p not in sys.path:
        sys.path.insert(0, _p)

import numpy as np

B, T, VOCAB, EMB, HID, OUT = 256, 2048, 7, 20, 128, 6
NCORES = 8
VK = VOCAB + 1          # vocab + identity pseudo-token for core 0 burn-in
BURN = int(os.environ.get("V6_BURN", "16"))
                        # burn-in rounds per chain (zero-state handoff error
                        # at 16 steps measured 1.6e-4 -- far below the bf16
                        # chain noise ~1.3e-3 and the 2e-2 tolerance)
NCH = 2                 # chains per core
SEG = T // NCORES       # 256 output steps per core
SEG2 = SEG // NCH       # 128 output steps per chain
TT = SEG2 + BURN        # 144 rounds
FCW = 16                # rounds per fc PSUM window ([128, 4*16*6] f32)
OG = 4                  # rounds per one-hot DMA chunk
NFILL = int(os.environ.get("V6_FILL", "3"))
GI, GF, GG, GO = 0, 1, 2, 3  # PyTorch gate order in W_hh rows / table cols


def _split_overloaded_waits(nc, mybir, max_other=1):
    """walrus in this env rejects instructions with more than a couple of sem
    waits (and InstDrain with any). Move excess waits onto same-engine NoOps
    emitted just before; same-engine program order preserves semantics."""
    n_split = 0
    for f in nc.m.functions:
        for blk in f.blocks:
            out = []
            changed = False
            for inst in blk.instructions:
                si = inst.sync_info
                waits = list(si.on_wait) if si is not None and si.on_wait else []
                limit = 0 if isinstance(inst, mybir.InstDrain) else max_other
                if len(waits) > limit:
                    moved = waits if limit == 0 else waits[limit:]
                    keep = [] if limit == 0 else waits[:limit]
                    for i0, w in enumerate(moved):
                        nop = mybir.InstNoOp(
                            name=f"{inst.name}-wsplit{i0}", ins=[], outs=[]
                        )
                        nop.engine = inst.engine
                        nop.sync_info = mybir.SyncInfo(on_wait=[w], on_update=[])
                        out.append(nop)
                        n_split += 1
                    inst.sync_info = mybir.SyncInfo(
                        on_wait=keep,
                        on_update=list(si.on_update) if si.on_update else [],
                    )
                    changed = True
                out.append(inst)
            if changed:
                blk.instructions = out
    return n_split


def _patch_tile_drain():
    import concourse.tile as tile
    from concourse.vector_clock import ScopedClock, VectorClock

    def _drain_and_barrier_split(self, tick_clock, wait_clock):
        gc = tick_clock.global_clock
        n = len(gc)
        for j in range(n):
            if gc[j] <= 0:
                continue
            vec = [0] * n
            vec[j] = gc[j]
            nop = self.nc.sync.nop(nofuse=True, hint=f"drain_split_{j}")
            wait_clock.add_sem_waits(nop.ins, ScopedClock({None: VectorClock(vec)}))
        self.nc.sync.drain()
        self.nc.all_engine_barrier()
        assert self.sems is not None
        popped = self.nc._tile_sem_poison_stack.pop()
        assert popped is self._sem_poison
        self.nc.clear_and_free_semaphores(list(self.sems.allocated().values()))
        self.nc.all_engine_barrier()

    tile.TileContext._drain_and_barrier = _drain_and_barrier_split


_BUILD_CACHE = {}


def _build_nc(for_sim=False):
    key = (TT, for_sim, NFILL)
    if key in _BUILD_CACHE:
        return _BUILD_CACHE[key]
    import concourse.bass as bass
    import concourse.mybir as mybir
    import concourse.tile as tile

    _patch_tile_drain()

    f32 = mybir.dt.float32
    bf16 = mybir.dt.bfloat16
    AF = mybir.ActivationFunctionType
    ALU = mybir.AluOpType

    nc = bass.Bass("TRN2", target_bir_lowering=False, debug=False)
    d_oh = nc.dram_tensor("onehot", [VK, TT * NCH * B], bf16, kind="ExternalInput")
    d_c0 = nc.dram_tensor("c0T", [HID, NCH * B], bf16, kind="ExternalInput")
    d_w = nc.dram_tensor("w", [HID, 4 * HID], bf16, kind="ExternalInput")
    d_tbl = nc.dram_tensor("tbl", [VK, 4 * HID], bf16, kind="ExternalInput")
    d_wfc = nc.dram_tensor("wfc", [HID, OUT], bf16, kind="ExternalInput")
    d_bfcw = nc.dram_tensor("bfcw", [1, 4 * FCW * OUT], bf16, kind="ExternalInput")
    d_out = nc.dram_tensor("out", [HID, NCH * SEG2 * NCH, OUT], f32,
                           kind="ExternalOutput")

    NWIN = SEG2 // FCW  # 8 fc windows; softmax windows align 1:1

    with tile.TileContext(nc) as tc, tc.tile_pool(name="const", bufs=1) as constp:
        w_sb = constp.tile([HID, 4 * HID], bf16, name="w_sb")
        tbl_sb = constp.tile([VK, 4 * HID], bf16, name="tbl_sb")
        wfc_sb = constp.tile([HID, OUT], bf16, name="wfc_sb")
        bfcw_sb = constp.tile([1, 4 * FCW * OUT], bf16, name="bfcw_sb")
        ones_sb = constp.tile([1, HID], bf16, name="ones_sb")
        cst2 = constp.tile([HID, NCH * B], bf16, name="cst2")
        h0_sb = constp.tile([HID, B], bf16, name="h0_sb")
        scr = constp.tile([HID, B], bf16, name="scr")
        fillsrc = constp.tile([HID, 2 * B], bf16, name="fillsrc")
        logit_sb = constp.tile([HID, NCH * SEG2 * NCH * OUT], f32, name="logit_sb")
        den_sb = constp.tile([HID, NCH * SEG2 * NCH], f32, name="den_sb")

        nc.sync.dma_start(w_sb[:], d_w.ap())
        nc.sync.dma_start(tbl_sb[:], d_tbl.ap())
        nc.sync.dma_start(wfc_sb[:], d_wfc.ap())
        nc.sync.dma_start(bfcw_sb[:], d_bfcw.ap())
        nc.sync.dma_start(cst2[:], d_c0.ap())
        nc.vector.memset(h0_sb[:], 0.0)
        nc.vector.memset(ones_sb[:], 1.0)
        nc.vector.memset(fillsrc[:], 0.0)
        # Pin the sigmoid_and_others table (contains tanh too) before the loop.
        nc.scalar.activation(scr[:], h0_sb[:], AF.Sigmoid)

        cst = [cst2[:, c * B : (c + 1) * B] for c in range(NCH)]

        with (
            tc.tile_pool(name="ohp", bufs=3) as ohp,
            tc.tile_pool(name="gatep", bufs=1, space="PSUM") as gatep,
            tc.tile_pool(name="fcp", bufs=1, space="PSUM") as fcp,
            tc.tile_pool(name="fillp", bufs=1, space="PSUM") as fillp,
            tc.tile_pool(name="ringp", bufs=3) as ringp,
            tc.tile_pool(name="workp", bufs=2) as workp,
        ):
            oh_tiles = [None] * (TT // OG + 1)

            def fetch_oh(chunk):
                ohc = ohp.tile([VK, OG * NCH * B], bf16, tag="oh")
                nc.sync.dma_start(
                    ohc[:],
                    d_oh.ap()[:, chunk * OG * NCH * B : (chunk + 1) * OG * NCH * B],
                )
                oh_tiles[chunk] = ohc

            fetch_oh(0)

            mergesig = os.environ.get("V6_MERGESIG", "1") == "1"

            def alloc_pair(r, c):
                """Allocate chain c's round-r gate PSUM (pair of banks, or one
                2-bank tile in mergesig mode) and prefill from the one-hot
                block (4 matmuls, K=VK, N=256). Quarter order [2g|i|f|o]."""
                if mergesig:
                    psAB = gatep.tile([128, 4 * B], f32, tag=f"psAB{c}")
                    quarters = [psAB[:, j * B : (j + 1) * B] for j in range(4)]
                else:
                    psA = gatep.tile([128, 2 * B], f32, tag=f"psA{c}")
                    psB = gatep.tile([128, 2 * B], f32, tag=f"psB{c}")
                    quarters = [
                        psA[:, 0:B], psA[:, B : 2 * B],
                        psB[:, 0:B], psB[:, B : 2 * B],
                    ]
                oh = oh_tiles[r // OG]
                col = ((r % OG) * NCH + c) * B
                for j, q in enumerate((GG, GI, GF, GO)):
                    nc.tensor.matmul(
                        quarters[j],
                        tbl_sb[:, q * HID : (q + 1) * HID],
                        oh[:, col : col + B],
                        start=(j % 2 == 0),
                        stop=False,
                        skip_group_check=True,
                    )
                if mergesig:
                    return (psAB, quarters)
                return (psA, psB, quarters)

            fill_ps = fillp.tile([128, 2 * B], f32, name="fill_ps")
            cur = [alloc_pair(0, 0), alloc_pair(0, 1)]
            nxt = [None, None]
            fcw_box = [None]
            pending_fc = [None, None]
            h_prev = [h0_sb[:], h0_sb[:]]

            def fc_round(t, order):
                """fc for output step t (both chains). Shared window bank
                [128, (chain, half, FCW, OUT)] f32; evacuated by GPSIMD."""
                w0 = t % FCW
                if w0 == 0:
                    fcw_box[0] = fcp.tile([HID, 4 * FCW * OUT], f32, tag="fcw", name="fcw")
                    nc.tensor.matmul(
                        fcw_box[0][:], ones_sb[:], bfcw_sb[:],
                        start=True, stop=False, skip_group_check=True,
                    )
                fcw = fcw_box[0]
                last_c = order[-1]
                for c in order:
                    for hf in range(2):
                        o0 = (((c * 2) + hf) * FCW + w0) * OUT
                        nc.tensor.matmul(
                            fcw[:, o0 : o0 + OUT],
                            pending_fc[c][:, hf * HID : (hf + 1) * HID],
                            wfc_sb[:],
                            start=False,
                            stop=(w0 == FCW - 1 and hf == 1 and c == last_c),
                            skip_group_check=True,
                        )
                    pending_fc[c] = None
                if w0 == FCW - 1:
                    win = t // FCW
                    nc.scalar.copy(
                        logit_sb[:, win * 4 * FCW * OUT : (win + 1) * 4 * FCW * OUT],
                        fcw[:],
                    )

            tanh_early = os.environ.get("V6_TANH_EARLY", "1") == "1"
            for r in range(TT):
                order = (0, 1) if r % 2 == 0 else (1, 0)
                a, c_b = order
                # --- PE: gate matmuls (critical), then fc, fillers
                for c in order:
                    quarters = cur[c][-1]
                    for j, q in enumerate((GG, GI, GF, GO)):
                        nc.tensor.matmul(
                            quarters[j],
                            w_sb[:, q * HID : (q + 1) * HID],
                            h_prev[c], start=False, stop=(j % 2 == 1),
                            skip_group_check=True,
                        )
                if pending_fc[0] is not None:
                    fc_round(r - 1 - BURN, order)
                for _ in range(NFILL):
                    nc.tensor.matmul(
                        fill_ps[:, 0:B], w_sb[:, 0:HID], scr[:],
                        start=True, stop=True, skip_group_check=True,
                    )
                # --- ACT sigmoids + DVE cell updates, interleaved so the
                # FIFO queues drain in readiness order. a = lead chain.
                sgi = [None, None]
                sfo = [None, None]
                tg = [None, None]
                ig = [None, None]
                mmb = [None, None]
                tcl = [None, None]
                hsl = [None, None]

                def act_sgi(c):
                    if mergesig:
                        sgi[c] = workp.tile(
                            [HID, 4 * B], bf16, tag=f"sgi{c}", name=f"sgi{c}"
                        )
                        nc.scalar.activation(sgi[c][:], cur[c][0][:], AF.Sigmoid)
                        sfo[c] = sgi[c]
                    else:
                        sgi[c] = workp.tile(
                            [HID, 2 * B], bf16, tag=f"sgi{c}", name=f"sgi{c}"
                        )
                        nc.scalar.activation(sgi[c][:], cur[c][0][:], AF.Sigmoid)

                def act_sfo(c):
                    if mergesig:
                        return
                    sfo[c] = workp.tile([HID, 2 * B], bf16, tag=f"sfo{c}", name=f"sfo{c}")
                    nc.scalar.activation(sfo[c][:], cur[c][1][:], AF.Sigmoid)

                def dve_head(c):
                    tg[c] = workp.tile([HID, B], bf16, tag=f"tg{c}", name=f"tg{c}")
                    ig[c] = workp.tile([HID, B], bf16, tag=f"ig{c}", name=f"ig{c}")
                    nc.vector.tensor_scalar(
                        tg[c][:], sgi[c][:, 0:B], 2.0, 1.0,
                        op0=ALU.mult, op1=ALU.subtract,
                    )
                    nc.vector.tensor_mul(ig[c][:], tg[c][:], sgi[c][:, B : 2 * B])

                def dve_cell(c):
                    sf_off = 2 * B if mergesig else 0
                    mmb[c] = workp.tile([HID, B], bf16, tag=f"mm{c}", name=f"mm{c}")
                    nc.vector.tensor_mul(
                        mmb[c][:], sfo[c][:, sf_off : sf_off + B], cst[c]
                    )
                    nc.vector.tensor_add(cst[c], mmb[c][:], ig[c][:])

                def act_tanh(c):
                    tcl[c] = workp.tile([HID, B], bf16, tag=f"tcl{c}", name=f"tcl{c}")
                    nc.scalar.activation(tcl[c][:], cst[c], AF.Tanh)

                def dve_h(c):
                    so_off = 3 * B if mergesig else B
                    hsl[c] = ringp.tile([HID, B], bf16, tag=f"h{c}", name=f"h{c}")
                    nc.vector.tensor_mul(
                        hsl[c][:], sfo[c][:, so_off : so_off + B], tcl[c][:]
                    )

                if tanh_early:
                    act_sgi(a)
                    act_sfo(a)
                    act_sgi(c_b)
                    dve_head(a)
                    dve_cell(a)
                    act_tanh(a)
                    act_sfo(c_b)
                    dve_head(c_b)
                    dve_h(a)
                    dve_cell(c_b)
                    act_tanh(c_b)
                    dve_h(c_b)
                else:
                    act_sgi(a)
                    act_sfo(a)
                    act_sgi(c_b)
                    act_sfo(c_b)
                    dve_head(a)
                    dve_cell(a)
                    act_tanh(a)
                    dve_head(c_b)
                    dve_h(a)
                    dve_cell(c_b)
                    act_tanh(c_b)
                    dve_h(c_b)

                # --- PE: prefill round r+1 (after the sigmoids that free the
                # recycled banks are emitted — bufs=1 WAR ordering)
                if (r + 1) % OG == 0 and (r + 1) // OG < TT // OG:
                    fetch_oh((r + 1) // OG)
                if r + 1 < TT:
                    for c in order:
                        nxt[c] = alloc_pair(r + 1, c)

                for c in (0, 1):
                    h_prev[c] = hsl[c][:]
                    if r >= BURN:
                        pending_fc[c] = hsl[c][:]
                    if r + 1 < TT:
                        cur[c] = nxt[c]
            fc_round(SEG2 - 1, (0, 1))

        # ---- phase 2: softmax over OUT, windowed; layout is
        # [128, (win, chain, half, w, OUT)] and the host fixes the order. ----
        CDIM = NCH * SEG2 * NCH  # 512 (row groups of OUT)
        p3 = logit_sb[:].rearrange("p (c o) -> p c o", o=OUT)
        NW = 8
        q = CDIM // NW
        for k in range(NW):
            lo, hi = k * q, (k + 1) * q
            nc.scalar.activation(
                logit_sb[:, lo * OUT : hi * OUT],
                logit_sb[:, lo * OUT : hi * OUT], AF.Exp,
            )
            nc.vector.reduce_sum(
                den_sb[:, lo:hi], p3[:, lo:hi, :], axis=mybir.AxisListType.X
            )
            nc.vector.reciprocal(den_sb[:, lo:hi], den_sb[:, lo:hi])
            rec_b = den_sb[:, lo:hi].unsqueeze(2).broadcast_to([HID, q, OUT])
            nc.vector.tensor_mul(p3[:, lo:hi, :], p3[:, lo:hi, :], rec_b)
            nc.sync.dma_start(d_out.ap()[:, lo:hi, :], p3[:, lo:hi, :])

    if not for_sim:
        _split_overloaded_waits(nc, mybir)
    _BUILD_CACHE[key] = nc
    return nc


def _host_prep(inputs, c0, W_ih, W_hh, b_ih, b_hh, W_fc, b_fc, emb):
    import ml_dtypes

    bf16 = ml_dtypes.bfloat16
    inputs = np.asarray(inputs)
    table_f = np.zeros((VK, 4 * HID), np.float32)
    table_f[:VOCAB] = emb @ W_ih.T + (b_ih + b_hh)
    table_f[VOCAB, GI * HID : (GI + 1) * HID] = -30.0
    table_f[VOCAB, GF * HID : (GF + 1) * HID] = 30.0
    table_f[VOCAB, GO * HID : (GO + 1) * HID] = -30.0
    w_f = W_hh.T.copy()
    table_f[:, GG * HID : (GG + 1) * HID] *= 2.0
    w_f[:, GG * HID : (GG + 1) * HID] *= 2.0
    table = table_f.astype(bf16)
    w = np.ascontiguousarray(w_f.astype(bf16))
    wfc = np.ascontiguousarray(W_fc.T.astype(bf16))
    bfcw = np.ascontiguousarray(
        np.tile(b_fc.astype(bf16), 4 * FCW).reshape(1, 4 * FCW * OUT)
    )
    c0T = np.ascontiguousarray(c0[0].T.astype(bf16))
    in_maps = []
    for core in range(NCORES):
        s = core * SEG
        toks = []
        for c in range(NCH):
            st = s + c * SEG2
            if core == 0 and c == 0:
                tok = np.concatenate(
                    [np.full((B, BURN), VOCAB, np.int64), inputs[:, :SEG2]],
                    axis=1,
                )
            else:
                tok = inputs[:, st - BURN : st + SEG2]
            toks.append(tok.T)  # [TT, B]
        tok_all = np.stack(toks, axis=1)  # [TT, NCH, B]
        oh = np.zeros((VK, TT * NCH * B), dtype=bf16)
        cols = np.arange(TT * NCH * B)
        oh[tok_all.reshape(-1), cols] = 1.0
        cc = np.zeros((HID, NCH * B), dtype=bf16)
        if core == 0:
            cc[:, 0:B] = c0T
        in_maps.append(
            {
                "onehot": oh,
                "c0T": cc,
                "w": w,
                "tbl": table,
                "wfc": wfc,
                "bfcw": bfcw,
            }
        )
    return in_maps


def _run(inputs, c0, W_ih, W_hh, b_ih, b_hh, W_fc, b_fc, emb, trace=False):
    from concourse.bass_utils import run_bass_kernel_spmd

    nc = _build_nc()
    in_maps = _host_prep(inputs, c0, W_ih, W_hh, b_ih, b_hh, W_fc, b_fc, emb)
    res = run_bass_kernel_spmd(
        nc, in_maps, core_ids=list(range(NCORES)), trace=trace
    )
    NWIN = SEG2 // FCW
    segs = []
    for core in range(NCORES):
        arr = res.results[core]["out"]  # [128, 512, 6]
        arr = arr.reshape(HID, NWIN, NCH, 2, FCW, OUT)
        # batch = hf*128 + p ; t(core-rel) = c*SEG2 + win*FCW + w
        arr = arr.transpose(3, 0, 2, 1, 4, 5).reshape(B, SEG, OUT)
        segs.append(arr)
    out = np.concatenate(segs, axis=1)
    return out, res


def kernel(inputs, c0, W_ih, W_hh, b_ih, b_hh, W_fc, b_fc, emb):
    out, _ = _run(
        np.asarray(inputs), np.asarray(c0), np.asarray(W_ih), np.asarray(W_hh),
        np.asarray(b_ih), np.asarray(b_hh), np.asarray(W_fc), np.asarray(b_fc),
        np.asarray(emb),
    )
    return out
